# revision 88
# baseline (speedup 1.0000x reference)
"""GCATopo (2-layer GTAT GNN) Trainium2 kernel, 8-way SPMD — v2.

Strategy (v2 redesign vs v1):
 - Node-major aggregation: per 128-edge tile ONE 512-wide matmul
   (lhsT=St one-hot, rhs=et2-weighted gathered features) accumulates
   [dst, 512] in a single PSUM bank; softmax denominators aggregate in a
   second small matmul. Normalization becomes per-partition scaling.
 - Per-edge dst logits come from a lookup matmul (lhsT=StT, rhs=local
   per-block dst-attn rows) instead of a 256B-per-edge DMA gather.
 - All per-edge elementwise work (logits, leaky-relu, exp, message
   weighting) is batched across a block's ~14 tiles with strided 3D/4D
   APs — a handful of DVE/Act instructions per block instead of ~15 per
   tile.
 - L2's topo output is discarded by the model, so L2 ships only
   [feat 512 | ta 4] and skips the SM stream entirely.
 - All matmul operands bf16 (4x PE rate vs f32); weights are host-folded
   (wl@attB etc.) and host-transposed; x arrives pre-transposed bf16.
 - Biases are folded forward into the next layer's constant rows, so
   drains are pure scaling.
 - L2 prep is fused into the L1 edge-phase block loop (PE prep matmuls
   overlap DVE/DMA edge work).
"""

from contextlib import ExitStack

import ml_dtypes
import numpy as np

import concourse.bacc as bacc
import concourse.tile as tile
from concourse import mybir
from concourse.masks import make_identity
from concourse.bass_utils import run_bass_kernel_spmd

F32 = mybir.dt.float32
BF16 = mybir.dt.bfloat16
F8 = mybir.dt.float8e4
I16 = mybir.dt.int16
AF = mybir.ActivationFunctionType
OP = mybir.AluOpType

P = 128
BF = ml_dtypes.bfloat16


class Cfg:
    def __init__(self, N=20000, E=240000, FIN=576, HID=128, TOPO=15, H=4,
                 CORES=8, NEG=0.2):
        self.N, self.E, self.FIN, self.HID, self.TOPO, self.H = N, E, FIN, HID, TOPO, H
        self.CORES, self.NEG = CORES, NEG
        self.HC = H * HID                      # 512
        self.ROW = 768                         # gathered row: fp8 feat + pad
        self.RB = self.ROW // 2                # bf16 view width (384)
        self.NPC = N // CORES                  # nodes per core
        self.NBLK = (self.NPC + P - 1) // P    # dst blocks per core
        # aux slots within the BF16 VIEW of the row (bf16 element offsets;
        # feat occupies bf16-view [0:256))
        self.C_TOPO = 256                      # 256..270: topo (L1)
        self.C_ONE = 256 + TOPO                # 271: constant 1.0 (L1)
        self.C_AL = 272                        # 272..275: al (L1)
        self.C_TA = 276                        # 276..279: ta (L1)
        self.C_TA2 = 256                       # 256..259: ta (L2)


CFG = Cfg()
GT_MAX = 8  # max tiles (=128 idxs each) per gather call
GP_POOL_PCT = 25   # percent of Gp tiles offloaded DVE -> gpsimd (L1)
GP_POOL_PCT2 = 15  # same for L2 (gpsimd busier there)


def cdiv(a, b):
    return (a + b - 1) // b


def ktiles(F):
    return [(o, min(P, F - o)) for o in range(0, F, P)]


# --------------------------------------------------------------------------
# host-side graph preprocessing (pure indexing on edge_index)
# --------------------------------------------------------------------------

def host_prep(edge_index, cfg):
    N, CORES, NPC, NBLK = cfg.N, cfg.CORES, cfg.NPC, cfg.NBLK
    src = np.asarray(edge_index[0], dtype=np.int64)
    dst = np.asarray(edge_index[1], dtype=np.int64)
    loops = np.arange(N, dtype=np.int64)
    src = np.concatenate([src, loops])
    dst = np.concatenate([dst, loops])
    order = np.argsort(dst, kind="stable")
    s, d = src[order], dst[order]

    core_of = d // NPC
    blk_of = (d % NPC) // P
    counts = np.zeros((CORES, NBLK), dtype=np.int64)
    for c in range(CORES):
        m = core_of == c
        bb = blk_of[m]
        for b in range(NBLK):
            counts[c, b] = int((bb == b).sum())
    schedule = [max(1, cdiv(int(counts[:, b].max()), P)) for b in range(NBLK)]
    offs = np.concatenate([[0], np.cumsum(schedule)]).astype(np.int64)
    ttot = int(offs[-1])

    srcidx = np.zeros((CORES, ttot * P), dtype=np.int16)
    dstloc = np.full((CORES, ttot * P), -1.0, dtype=np.float32)
    for c in range(CORES):
        m = core_of == c
        sc, dc, bc = s[m], d[m], blk_of[m]
        for b in range(NBLK):
            mb = bc == b
            n = int(mb.sum())
            base = int(offs[b]) * P
            srcidx[c, base:base + n] = sc[mb].astype(np.int16)
            dstloc[c, base:base + n] = (dc[mb] - (c * NPC + b * P)).astype(np.float32)

    # wrap for dma_gather: index i lives at [i % 16, i // 16]; the 16-row
    # block is replicated 8x along partitions (one stripe per gpsimd core)
    src_w = [np.tile(srcidx[c].reshape(-1, 16).T, (8, 1)).copy()
             for c in range(CORES)]
    # host-built one-hot selection tables, per tile [St | StT] (bf16 0/1):
    #   St[e, d] = (dstloc[e] == d), StT = St^T
    rng = np.arange(P, dtype=np.float32)
    stt = []
    for c in range(CORES):
        dl = dstloc[c].reshape(ttot, P)
        St = (dl[:, :, None] == rng[None, None, :])          # [t, e, d]
        tab = np.concatenate([St, St.transpose(0, 2, 1)], 2)  # [t, p, 256]
        stt.append(np.ascontiguousarray(
            tab.transpose(1, 0, 2).reshape(P, ttot * 2 * P).astype(BF)))
    return schedule, src_w, stt


def host_weights(inputs, cfg):
    """All small-weight folding in f32 numpy, shipped as bf16."""
    H, C, TOPO, HC = cfg.H, cfg.HID, cfg.TOPO, cfg.HC
    f = lambda k: np.asarray(inputs[k], np.float32)

    def attB(att):  # [1,H,C] -> block-diag [H*C, H]
        out = np.zeros((H * C, H), np.float32)
        a = np.asarray(att, np.float32).reshape(H, C)
        for h in range(H):
            out[h * C:(h + 1) * C, h] = a[h]
        return out

    w = {}
    # topo extractor
    w["tw1"] = f("te_w1")                      # [576,128]
    w["tb1"] = f("te_b1").reshape(1, -1)
    w["tw2"] = f("te_w2")                      # [128,15]
    w["tb2"] = f("te_b2").reshape(1, -1)
    # layer 1
    aB1 = attB(inputs["l1_att"])
    w["wl1"] = f("l1_wl")                      # [576,512]
    w["bl1"] = f("l1_bl").reshape(1, -1)
    w["A1"] = np.concatenate([f("l1_wl") @ aB1, f("l1_wr") @ aB1], 1)  # [576,8]
    w["bA1"] = np.concatenate([f("l1_bl") @ aB1, f("l1_br") @ aB1]).reshape(1, -1)
    w["att2T1"] = f("l1_att2").reshape(H, TOPO).T      # [15,4]
    # layer 2 (input h1 = agg1_norm, l1_bias folded here)
    b1 = f("l1_bias")
    w["wl2"] = f("l2_wl")                      # [512,512]
    w["bl2"] = (b1 @ f("l2_wl") + f("l2_bl")).reshape(1, -1)
    w["att2T2"] = f("l2_att2").reshape(H, TOPO).T      # [15,4]
    # topo1 input to L2 = topo1_raw + l1_bias2; edge logit gets the const
    # twice (src+dst) -> fold 2*(b2@att2) into the dst-side rows only
    w["ta2c"] = (2.0 * (f("l1_bias2") @ w["att2T2"])).reshape(1, -1)   # [1,4]
    # heads (l2_bias folded into first-layer bias)
    b2f = f("l2_bias")
    for nm in ("v", "a"):
        w[f"{nm}w1"] = f(f"{nm}_w1")           # [512,128]
        w[f"{nm}b1"] = (f(f"{nm}_b1") + b2f @ f(f"{nm}_w1")).reshape(1, -1)
        w[f"{nm}w2"] = f(f"{nm}_w2")           # [128,1]
        w[f"{nm}b2"] = f(f"{nm}_b2").reshape(1, 1)
    # att2T2 flattened (h,j) row for the drain's ta2 reduce + const
    w["att2f"] = w["att2T2"].T.reshape(1, -1)  # [1,60] (h-major)
    return {k: v.astype(BF) for k, v in w.items()}


# --------------------------------------------------------------------------
# program builder
# --------------------------------------------------------------------------

class Prog:
    pass


def build_program(cfg, schedule, debug=False):
    es = ExitStack()
    nc = bacc.Bacc("TRN2", target_bir_lowering=False, debug=False,
                   num_devices=cfg.CORES)
    pr = Prog()
    pr.nc = nc
    N, FIN, HID, TOPO, H, HC, ROW, NPC, NBLK = (
        cfg.N, cfg.FIN, cfg.HID, cfg.TOPO, cfg.H, cfg.HC, cfg.ROW, cfg.NPC,
        cfg.NBLK)
    TTOT = sum(schedule)
    W16 = TTOT * P // 16
    groups = [list(range(cfg.CORES))]
    blocks = ktiles(NPC)
    fkt = ktiles(FIN)
    ckt = ktiles(HC)
    offs = np.concatenate([[0], np.cumsum(schedule)]).astype(int)

    def din(name, shape, dtype=BF16):
        return nc.dram_tensor(name, list(shape), dtype, kind="ExternalInput")

    # ---- external inputs ----
    xT = din("xT_slice", (FIN, NPC))
    wnames = [("tw1", (FIN, HID)), ("tb1", (1, HID)), ("tw2", (HID, TOPO)),
              ("tb2", (1, TOPO)), ("wl1", (FIN, HC)), ("bl1", (1, HC)),
              ("A1", (FIN, 2 * H)), ("bA1", (1, 2 * H)), ("att2T1", (TOPO, H)),
              ("wl2", (HC, HC)), ("bl2", (1, HC)), ("att2T2", (TOPO, H)),
              ("ta2c", (1, H)), ("att2f", (1, H * TOPO)),
              ("vw1", (HC, HID)), ("vb1", (1, HID)), ("vw2", (HID, 1)),
              ("vb2", (1, 1)),
              ("aw1", (HC, HID)), ("ab1", (1, HID)), ("aw2", (HID, 1)),
              ("ab2", (1, 1))]
    W = {nm: din(nm, sh) for nm, sh in wnames}
    src_i = din("src_idx", (P, W16), I16)
    stt_i = din("stt_tab", (P, TTOT * 2 * P))

    # ---- outputs ----
    val_o = nc.dram_tensor("valence", [1, 1], F32, kind="ExternalOutput")
    aro_o = nc.dram_tensor("arousal", [1, 1], F32, kind="ExternalOutput")
    dbg = {}
    if debug:
        for nm, sh in [("dbg_h1", (P, HC)), ("dbg_tt", (P, TOPO + H)),
                       ("dbg_psm1", (P, 68)), ("dbg_h2", (P, HC)),
                       ("dbg_pool", (P, H)), ("dbg_aux", (P, 24)),
                       ("dbg_psm2", (P, H)), ("dbg_pd1", (P, 2 * H))]:
            dbg[nm] = nc.dram_tensor(nm, list(sh), F32, kind="ExternalOutput")

    # ---- internal DRAM ----
    ext_sl = [nc.dram_tensor(f"ext_slice{L}", [NPC, ROW], F8)
              for L in (1, 2)]
    ext_fl = [nc.dram_tensor(f"ext_full{L}", [N, ROW], F8,
                             addr_space="Shared") for L in (1, 2)]
    pool_in = nc.dram_tensor("pool_in", [1, HC], F32)
    pool_out = nc.dram_tensor("pool_out", [1, HC], F32, addr_space="Shared")

    with tile.TileContext(nc) as tc:
        # ================= static SBUF =================
        ident = nc.alloc_sbuf_tensor("ident", [P, P], F32).ap()
        make_identity(nc, ident)
        ones_row = nc.alloc_sbuf_tensor("ones_row", [1, NPC], BF16).ap()
        nc.gpsimd.memset(ones_row, 1.0)
        ones_col = nc.alloc_sbuf_tensor("ones_col", [P, 1], BF16).ap()
        nc.gpsimd.memset(ones_col, 1.0)
        eps_col = nc.alloc_sbuf_tensor("eps_col", [P, 1], F32).ap()
        nc.gpsimd.memset(eps_col, 1e-30)

        src_sb = nc.alloc_sbuf_tensor("src_sb", [P, W16], I16).ap()
        nc.sync.dma_start(src_sb, src_i[:, :])

        # resident activations / weights
        xT_sb = [nc.alloc_sbuf_tensor(f"xT{i}", [P, NPC], BF16).ap()
                 for i in range(len(fkt))]
        for i, (fo, fk) in enumerate(fkt):
            nc.sync.dma_start(xT_sb[i][:fk, :], xT[fo:fo + fk, :])
        hfmT = [nc.alloc_sbuf_tensor(f"hfmT{i}", [P, NPC], BF16).ap()
                for i in range(len(ckt))]
        topoT0 = nc.alloc_sbuf_tensor("topoT0", [TOPO, NPC], BF16).ap()
        datt1 = nc.alloc_sbuf_tensor("datt1", [P, NBLK * 2 * H], BF16).ap()
        datt2 = nc.alloc_sbuf_tensor("datt2", [P, NBLK * H], BF16).ap()
        nc.vector.memset(datt1, 0.0)   # rows past a partial block stay 0
        nc.vector.memset(datt2, 0.0)

        wsb = {}
        for nm, sh in wnames:
            if sh[0] <= P:
                wsb[nm] = nc.alloc_sbuf_tensor(f"w_{nm}", list(sh), BF16).ap()
                nc.sync.dma_start(wsb[nm], W[nm][:, :])
            else:  # k-tiled along the first (contraction) dim
                tiles = []
                for i, (fo, fk) in enumerate(ktiles(sh[0])):
                    t = nc.alloc_sbuf_tensor(f"w_{nm}{i}", [fk, sh[1]],
                                             BF16).ap()
                    nc.sync.dma_start(t, W[nm][fo:fo + fk, :])
                    tiles.append(t)
                wsb[nm] = tiles
        # att2f / ta2c broadcast to all partitions
        att2bc = nc.alloc_sbuf_tensor("att2bc", [P, H * TOPO], BF16).ap()
        nc.gpsimd.partition_broadcast(att2bc, wsb["att2f"][0:1, :])
        ta2cbc = nc.alloc_sbuf_tensor("ta2cbc", [P, H], BF16).ap()
        nc.gpsimd.partition_broadcast(ta2cbc, wsb["ta2c"][0:1, :])
        ident_bf = nc.alloc_sbuf_tensor("ident_bf", [P, P], BF16).ap()
        nc.vector.tensor_copy(ident_bf, ident)

        # ================= phase A: topo MLP + L1 prep =================
        with tc.tile_pool(name="ppA", bufs=1, space="PSUM") as ppA, \
             tc.tile_pool(name="ppA2", bufs=2, space="PSUM") as ppA2, \
             tc.tile_pool(name="cpA", bufs=3) as cpA, \
             tc.tile_pool(name="spA", bufs=2) as spA:
            # --- topo extractor MLP (feat-major: out rows = hid/topo) ---
            NG = 512
            for go in range(0, NPC, NG):
                gs = min(NG, NPC - go)
                ph = ppA.tile([P, NG], F32, tag="ph", name="ph", space="PSUM")
                for i, (fo, fk) in enumerate(fkt):
                    nc.tensor.matmul(ph[:, :gs], lhsT=wsb["tw1"][i][:fk, :],
                                     rhs=xT_sb[i][:fk, go:go + gs],
                                     start=i == 0, stop=False,
                                     skip_group_check=True)
                nc.tensor.matmul(ph[:, :gs], lhsT=wsb["tb1"][:, :],
                                 rhs=ones_row[:, go:go + gs], start=False,
                                 stop=True, skip_group_check=True)
                t_hid = spA.tile([P, NG], BF16, tag="t_hid", name="t_hid")
                nc.scalar.activation(t_hid[:, :gs], ph[:, :gs], AF.Relu)
                pt = ppA.tile([TOPO, NG], F32, tag="pt", name="pt", space="PSUM")
                nc.tensor.matmul(pt[:, :gs], lhsT=wsb["tw2"][:, :],
                                 rhs=t_hid[:, :gs], start=True, stop=False,
                                 skip_group_check=True)
                nc.tensor.matmul(pt[:, :gs], lhsT=wsb["tb2"][:, :],
                                 rhs=ones_row[:, go:go + gs], start=False,
                                 stop=True, skip_group_check=True)
                nc.vector.tensor_copy(topoT0[:, go:go + gs], pt[:, :gs])

            # --- L1 prep per block ---
            for bi, (bo, bs) in enumerate(blocks):
                pm = ppA2.tile([P, HC], F32, tag="pm", name="pm", space="PSUM")
                pa = ppA.tile([P, 2 * H], F32, tag="pa", name="pa", space="PSUM")
                for i, (fo, fk) in enumerate(fkt):
                    nc.tensor.matmul(pm[:bs, :], lhsT=xT_sb[i][:fk, bo:bo + bs],
                                     rhs=wsb["wl1"][i][:fk, :],
                                     start=i == 0, stop=False,
                                     skip_group_check=True)
                    nc.tensor.matmul(pa[:bs, :], lhsT=xT_sb[i][:fk, bo:bo + bs],
                                     rhs=wsb["A1"][i][:fk, :],
                                     start=i == 0, stop=False,
                                     skip_group_check=True)
                nc.tensor.matmul(pm[:bs, :], lhsT=ones_row[:, bo:bo + bs],
                                 rhs=wsb["bl1"][:, :], start=False, stop=True,
                                 skip_group_check=True)
                nc.tensor.matmul(pa[:bs, :], lhsT=ones_row[:, bo:bo + bs],
                                 rhs=wsb["bA1"][:, :], start=False, stop=True,
                                 skip_group_check=True)
                pta = ppA.tile([P, H], F32, tag="pta", name="pta", space="PSUM")
                nc.tensor.matmul(pta[:bs, :], lhsT=topoT0[:, bo:bo + bs],
                                 rhs=wsb["att2T1"][:, :], start=True,
                                 stop=True, skip_group_check=True)
                ptt = ppA.tile([P, TOPO], BF16, tag="ptt", name="ptt",
                               space="PSUM")
                nc.tensor.transpose(ptt[:bs, :TOPO],
                                    topoT0[:, bo:bo + bs],
                                    ident_bf[:TOPO, :TOPO])
                ext = cpA.tile([P, ROW], F8, tag="ext", name="ext")
                extb = ext[:, :].bitcast(BF16)
                nc.scalar.copy(ext[:bs, 0:HC], pm[:bs, :])
                nc.scalar.copy(extb[:bs, cfg.C_TOPO:cfg.C_TOPO + TOPO],
                               ptt[:bs, :TOPO])
                nc.vector.memset(extb[:bs, cfg.C_ONE:cfg.C_ONE + 1], 1.0)
                nc.scalar.copy(extb[:bs, cfg.C_AL:cfg.C_AL + H], pa[:bs, 0:H])
                nc.scalar.copy(extb[:bs, cfg.C_TA:cfg.C_TA + H], pta[:bs, :])
                nc.sync.dma_start(ext_sl[0][bo:bo + bs, :], ext[:bs, :])
                if debug and bi == 0:
                    da = cpA.tile([P, 24], F32, tag="dbga", name="dbga")
                    nc.vector.tensor_copy(da[:, :],
                                          extb[:, cfg.C_TOPO:cfg.C_TOPO + 24])
                    nc.sync.dma_start(dbg["dbg_aux"][:, :], da[:, :])
                # dst-side rows: [ar | ta]
                nc.vector.tensor_copy(datt1[:bs, bi * 2 * H:bi * 2 * H + H],
                                      pa[:bs, H:2 * H])
                nc.vector.tensor_copy(
                    datt1[:bs, bi * 2 * H + H:(bi + 1) * 2 * H], pta[:bs, :])
            nc.gpsimd.collective_compute(
                "AllGather", OP.bypass, replica_groups=groups,
                ins=[ext_sl[0][:, :]], outs=[ext_fl[0][:, :]])

        # ================= edge phase (shared emitter) =================
        TMAX = max(schedule)

        def emit_edge(L, gp, sp, pp, pp2):
            AUXW = 2 * H if L == 1 else H      # lg width per tile
            AUXO = cfg.C_AL if L == 1 else cfg.C_TA2
            for bi, (bo, bs) in enumerate(blocks):
                Tb = schedule[bi]
                base = int(offs[bi])
                TW = Tb * P
                # ---- gathers ----
                G = gp.tile([P, TMAX * ROW], F8, tag="G", name="G")
                for go in range(0, Tb, GT_MAX):
                    gn = min(GT_MAX, Tb - go)
                    c0 = (base + go) * 8
                    nc.gpsimd.dma_gather(
                        G[:, go * ROW:(go + gn) * ROW].rearrange(
                            "p (t e) -> p t e", e=ROW),
                        ext_fl[L - 1][:, :], src_sb[:, c0:c0 + 8 * gn],
                        num_idxs=P * gn, num_idxs_reg=P * gn, elem_size=ROW,
                        queue_num=0)
                # ---- St / StT (host-built one-hot tables) ----
                stt = sp.tile([P, TMAX * 2 * P], BF16, tag="stt", name="stt")
                nc.sync.dma_start(stt[:, 0:Tb * 2 * P],
                                  stt_i[:, base * 2 * P:(base + Tb) * 2 * P])

                def St(t):
                    return stt[:, t * 2 * P:t * 2 * P + P]

                def StT(t):
                    return stt[:, t * 2 * P + P:(t + 1) * 2 * P]
                # ---- dst-logit lookup ----
                pD = pp.tile([P, TMAX * AUXW], F32, tag="pD", name="pD",
                             space="PSUM")
                dsl = (datt1[:, bi * 2 * H:(bi + 1) * 2 * H] if L == 1
                       else datt2[:, bi * H:(bi + 1) * H])
                for t in range(Tb):
                    nc.tensor.matmul(pD[:, t * AUXW:(t + 1) * AUXW],
                                     lhsT=StT(t), rhs=dsl, start=True,
                                     stop=True, skip_group_check=True)
                # ---- batched logits ----
                Gb = G[:, 0:Tb * ROW].bitcast(BF16).rearrange(
                    "p (t e) -> p t e", e=cfg.RB)
                lg = sp.tile([P, TMAX * AUXW], F32, tag="lg", name="lg")
                nc.vector.tensor_tensor(
                    lg[:, 0:Tb * AUXW].rearrange("p (t c) -> p t c", c=AUXW),
                    Gb[:, :, AUXO:AUXO + AUXW],
                    pD[:, 0:Tb * AUXW].rearrange("p (t c) -> p t c", c=AUXW),
                    OP.add)
                lr = sp.tile([P, TMAX * AUXW], F32, tag="lr", name="lr")
                nc.vector.scalar_tensor_tensor(
                    lr[:, 0:Tb * AUXW], lg[:, 0:Tb * AUXW], cfg.NEG,
                    lg[:, 0:Tb * AUXW], OP.mult, OP.max)
                et = sp.tile([P, TMAX * AUXW], BF16, tag="et", name="et")
                nc.scalar.activation(et[:, 0:Tb * AUXW], lr[:, 0:Tb * AUXW],
                                     AF.Exp)
                etv = et[:, 0:Tb * AUXW].rearrange("p (t c) -> p t c", c=AUXW)
                # ---- weighted messages (split DVE / gpsimd) ----
                Gp = gp.tile([P, TMAX * HC], BF16, tag="Gp", name="Gp")
                e2off = H if L == 1 else 0
                Gf = G[:, 0:Tb * ROW].rearrange("p (t e) -> p t e", e=ROW)
                pct = GP_POOL_PCT if L == 1 else GP_POOL_PCT2
                ks = (Tb * pct + 99) // 100      # first ks tiles on Pool

                def gp_op(eng, t0, t1):
                    if t1 <= t0:
                        return
                    eng.tensor_tensor(
                        Gp[:, t0 * HC:t1 * HC].rearrange(
                            "p (t h c) -> p t h c", h=H, c=HID),
                        Gf[:, t0:t1, 0:HC].rearrange(
                            "p t (h c) -> p t h c", c=HID),
                        etv[:, t0:t1, e2off:e2off + H].unsqueeze(
                            3).to_broadcast((P, t1 - t0, H, HID)),
                        OP.mult)
                for t0 in range(0, ks, 2):        # gpsimd in 2-tile chunks
                    gp_op(nc.gpsimd, t0, min(t0 + 2, ks))
                for t0 in range(ks, Tb, 3):       # DVE in 3-tile chunks
                    gp_op(nc.vector, t0, min(t0 + 3, Tb))
                if L == 1:
                    SMW = 16 * H + H
                    SMe = sp.tile([P, TMAX * SMW], BF16, tag="SMe", name="SMe")
                    SMv = SMe[:, 0:Tb * SMW].rearrange("p (t c) -> p t c",
                                                       c=SMW)
                    nc.vector.tensor_tensor(
                        SMv[:, :, 0:16 * H].rearrange(
                            "p t (h j) -> p t h j", j=16),
                        Gb[:, :, cfg.C_TOPO:cfg.C_TOPO + 16].unsqueeze(
                            2).to_broadcast((P, Tb, H, 16)),
                        etv[:, :, 0:H].unsqueeze(3).to_broadcast(
                            (P, Tb, H, 16)),
                        OP.mult)
                    nc.scalar.copy(SMv[:, :, 16 * H:SMW],
                                   etv[:, :, H:2 * H])
                else:
                    SMW = H
                    SMe = et
                # ---- aggregation matmuls ----
                pf = pp2.tile([P, HC], F32, tag="pf", name="pf", space="PSUM")
                psm = pp2.tile([P, SMW], F32, tag="psm", name="psm",
                               space="PSUM")
                for t in range(Tb):
                    st0, sp1 = t == 0, t == Tb - 1
                    nc.tensor.matmul(pf[:, :], lhsT=St(t),
                                     rhs=Gp[:, t * HC:(t + 1) * HC],
                                     start=st0, stop=sp1,
                                     skip_group_check=True)
                    nc.tensor.matmul(psm[:, :], lhsT=St(t),
                                     rhs=SMe[:, t * SMW:(t + 1) * SMW],
                                     start=st0, stop=sp1,
                                     skip_group_check=True)
                if debug and bi == 0:
                    dt = sp.tile([P, 68], F32, tag="dbgp", name="dbgp")
                    nc.vector.tensor_copy(dt[:, 0:SMW], psm[:, :])
                    nc.sync.dma_start(
                        dbg["dbg_psm1" if L == 1 else "dbg_psm2"][:, 0:SMW],
                        dt[:, 0:SMW])
                    dp = sp.tile([P, 2 * H], F32, tag="dbgd", name="dbgd")
                    nc.vector.tensor_copy(dp[:, 0:AUXW], pD[:, 0:AUXW])
                    if L == 1:
                        nc.sync.dma_start(dbg["dbg_pd1"][:, 0:AUXW],
                                          dp[:, 0:AUXW])
                # ---- drain ----
                if L == 1:
                    drain1(bi, bo, bs, pf, psm, sp, pp, pp2)
                else:
                    drain2(bi, bo, bs, pf, psm, sp, pp)

        # ---- L1 drain + fused L2 prep ----
        def drain1(bi, bo, bs, pf, psm, sp, pp, pp2):
            # rec2 = 1/sum(e2), rec1' = 1/(H*sum(e1))
            den = sp.tile([P, 2 * H], F32, tag="den", name="den")
            nc.vector.tensor_scalar(
                den[:, 0:H].unsqueeze(2),
                psm[:, 0:16 * H].rearrange("p (h j) -> p h j", j=16)[
                    :, :, 15:16],
                float(H), eps_col[:, 0:1], OP.mult, OP.max)
            nc.vector.tensor_tensor(den[:, H:2 * H], psm[:, 16 * H:16 * H + H],
                                    eps_col[:, 0:1].to_broadcast((P, H)),
                                    OP.max)
            rec = sp.tile([P, 2 * H], F32, tag="rec", name="rec")
            nc.vector.reciprocal(rec[:, :], den[:, :])
            # h1 = agg_feat * rec2 (node-major, bf16; per-head scale on Act)
            h1 = sp.tile([P, HC], BF16, tag="h1", name="h1")
            for h in range(H):
                nc.scalar.activation(h1[:, h * HID:(h + 1) * HID],
                                     pf[:, h * HID:(h + 1) * HID], AF.Copy,
                                     scale=rec[:, H + h:H + h + 1])
            # topo1_raw = sum_h agg_topo_h * rec1'   [d, 15]
            tp = sp.tile([P, TOPO * H], F32, tag="tp", name="tp")
            nc.vector.tensor_tensor(
                tp[:, :].rearrange("p (j h) -> p j h", h=H),
                psm[:, 0:16 * H].rearrange("p (h j) -> p h j", j=16)[
                    :, :, 0:TOPO].transpose([0, 2, 1]),
                rec[:, 0:H].unsqueeze(1).to_broadcast((P, TOPO, H)),
                OP.mult)
            t1 = sp.tile([P, TOPO], F32, tag="t1", name="t1")
            nc.vector.tensor_reduce(
                t1[:, :], tp[:, :].rearrange("p (j h) -> p j h", h=H),
                mybir.AxisListType.X, OP.add)
            # ta2 = topo1_raw @ att2T2 (per-node, via DVE reduce)
            tq = sp.tile([P, H * TOPO], F32, tag="tq", name="tq")
            nc.vector.tensor_tensor(
                tq[:, :].rearrange("p (h j) -> p h j", j=TOPO),
                t1[:, :].unsqueeze(1).to_broadcast((P, H, TOPO)),
                att2bc[:, :].rearrange("p (h j) -> p h j", j=TOPO),
                OP.mult)
            ta2 = sp.tile([P, H], F32, tag="ta2", name="ta2")
            nc.vector.tensor_reduce(
                ta2[:, :], tq[:, :].rearrange("p (h j) -> p h j", j=TOPO),
                mybir.AxisListType.X, OP.add)
            # dst rows for L2: ta2 + 2*(b2@att2)
            nc.vector.tensor_tensor(datt2[:bs, bi * H:(bi + 1) * H],
                                    ta2[:bs, :],
                                    ta2cbc[:bs, :], OP.add)
            if debug and bi == 0:
                dh = sp.tile([P, HC], F32, tag="dbgh", name="dbgh")
                nc.vector.tensor_copy(dh[:, :], h1[:, :])
                nc.sync.dma_start(dbg["dbg_h1"][:, :], dh[:, :])
                dtt = sp.tile([P, TOPO + H], F32, tag="dbgt", name="dbgt")
                nc.vector.tensor_copy(dtt[:, 0:TOPO], t1[:, :])
                nc.vector.tensor_copy(dtt[:, TOPO:TOPO + H], ta2[:, :])
                nc.sync.dma_start(dbg["dbg_tt"][:, :], dtt[:, :])
            # transpose h1 -> hfmT tiles
            for ci, (co, ck) in enumerate(ckt):
                ptr = pp.tile([P, P], BF16, tag="ptr", name="ptr",
                              space="PSUM")
                nc.tensor.transpose(ptr[:ck, :bs], h1[:bs, co:co + ck],
                                    ident_bf[:bs, :bs])
                nc.scalar.copy(hfmT[ci][:ck, bo:bo + bs], ptr[:ck, :bs])
            # ---- fused L2 prep for this block ----
            pm2 = pp2.tile([P, HC], F32, tag="pm2", name="pm2", space="PSUM")
            for ci, (co, ck) in enumerate(ckt):
                nc.tensor.matmul(pm2[:bs, :], lhsT=hfmT[ci][:ck, bo:bo + bs],
                                 rhs=wsb["wl2"][ci][:ck, :],
                                 start=ci == 0, stop=False,
                                 skip_group_check=True)
            nc.tensor.matmul(pm2[:bs, :], lhsT=ones_row[:, bo:bo + bs],
                             rhs=wsb["bl2"][:, :], start=False, stop=True,
                             skip_group_check=True)
            ext = sp.tile([P, ROW], F8, tag="ext2", name="ext2")
            nc.scalar.copy(ext[:bs, 0:HC], pm2[:bs, :])
            nc.scalar.copy(ext[:, :].bitcast(BF16)[
                :bs, cfg.C_TA2:cfg.C_TA2 + H], ta2[:bs, :])
            nc.sync.dma_start(ext_sl[1][bo:bo + bs, :], ext[:bs, :])

        # ---- L2 drain: normalize + pooled partial ----
        def drain2(bi, bo, bs, pf, psm, sp, pp):
            den = sp.tile([P, H], F32, tag="den2", name="den2")
            nc.vector.tensor_tensor(den[:, :], psm[:, 0:H],
                                    eps_col[:, 0:1].to_broadcast((P, H)),
                                    OP.max)
            rec = sp.tile([P, H], F32, tag="rec2", name="rec2")
            nc.vector.reciprocal(rec[:, :], den[:, :])
            h2 = sp.tile([P, HC], BF16, tag="h2", name="h2")
            for h in range(H):
                nc.scalar.activation(h2[:, h * HID:(h + 1) * HID],
                                     pf[:, h * HID:(h + 1) * HID], AF.Copy,
                                     scale=rec[:, h:h + 1])
            if debug and bi == 0:
                dh = sp.tile([P, HC], F32, tag="dbgh2", name="dbgh2")
                nc.vector.tensor_copy(dh[:, :], h2[:, :])
                nc.sync.dma_start(dbg["dbg_h2"][:, :], dh[:, :])
            nc.tensor.matmul(pr.pool_ps[:, :], lhsT=ones_col[:bs, 0:1],
                             rhs=h2[:bs, :], start=bi == 0,
                             stop=bi == len(blocks) - 1,
                             skip_group_check=True)

        # ================= phase B/C: L1 edges (+L2 prep) =================
        # PSUM banks: (pf+psm) 2x2 + pm2 2 + pD/ptr 1 each = 8 of 8
        with tc.tile_pool(name="gpB", bufs=3) as gpB, \
             tc.tile_pool(name="spB", bufs=3) as spB, \
             tc.tile_pool(name="ppB", bufs=1, space="PSUM") as ppB, \
             tc.tile_pool(name="ppB2", bufs=2, space="PSUM") as ppB2:
            emit_edge(1, gpB, spB, ppB, ppB2)
            nc.gpsimd.collective_compute(
                "AllGather", OP.bypass, replica_groups=groups,
                ins=[ext_sl[1][:, :]], outs=[ext_fl[1][:, :]])

        # ================= phase D: L2 edges =================
        with tc.tile_pool(name="gpD", bufs=3) as gpD, \
             tc.tile_pool(name="spD", bufs=3) as spD, \
             tc.tile_pool(name="ppD", bufs=1, space="PSUM") as ppD, \
             tc.tile_pool(name="ppD2", bufs=2, space="PSUM") as ppD2, \
             tc.tile_pool(name="plD", bufs=1, space="PSUM") as plD:
            pr.pool_ps = plD.tile([1, HC], F32, tag="pool", name="pool",
                                  space="PSUM", bufs=1)
            emit_edge(2, gpD, spD, ppD, ppD2)

            # ---- pool + heads ----
            pooled = spD.tile([1, HC], F32, tag="pooled", name="pooled")
            nc.vector.tensor_copy(pooled[:, :], pr.pool_ps[:, :])
            nc.sync.dma_start(pool_in[:, :], pooled[:, :])
            nc.gpsimd.collective_compute(
                "AllReduce", OP.add, replica_groups=groups,
                ins=[pool_in[:, :]], outs=[pool_out[:, :]])
            # load back column-major: pmean_cols[c, h] = pool_out[h*HID+c]
            pooled2 = spD.tile([P, H], F32, tag="pooled2", name="pooled2")
            with nc.allow_non_contiguous_dma("pool row -> col-major reload"):
                nc.sync.dma_start(
                    pooled2[:, :],
                    pool_out[:, :].rearrange("o (h c) -> (o c) h", c=HID))
            if debug:
                nc.sync.dma_start(dbg["dbg_pool"][:, :], pooled2[:, :])
            pmean = spD.tile([P, H], BF16, tag="pmean", name="pmean")
            nc.vector.tensor_scalar(pmean[:, :], pooled2[:, :], 1.0 / N,
                                    None, OP.mult)
            for nm, out_t in (("v", val_o), ("a", aro_o)):
                pm = ppD.tile([P, 1], F32, tag="mlp", name="mlp", space="PSUM")
                for ki in range(H):
                    nc.tensor.matmul(pm[:, :], lhsT=wsb[f"{nm}w1"][ki][:, :],
                                     rhs=pmean[:, ki:ki + 1], start=ki == 0,
                                     stop=False, skip_group_check=True)
                nc.tensor.matmul(pm[:, :], lhsT=wsb[f"{nm}b1"][:, :],
                                 rhs=ones_col[0:1, :], start=False, stop=True,
                                 skip_group_check=True)
                hv = spD.tile([P, 1], BF16, tag=f"{nm}hv", name=f"{nm}hv")
                nc.scalar.activation(hv[:, :], pm[:, :], AF.Relu)
                po = ppD.tile([1, 1], F32, tag="mlpo", name="mlpo",
                              space="PSUM")
                nc.tensor.matmul(po[:, :], lhsT=hv[:, :],
                                 rhs=wsb[f"{nm}w2"][:, :], start=True,
                                 stop=False, skip_group_check=True)
                nc.tensor.matmul(po[:, :], lhsT=wsb[f"{nm}b2"][:, :],
                                 rhs=ones_col[0:1, :], start=False, stop=True,
                                 skip_group_check=True)
                ov = spD.tile([1, 1], F32, tag=f"{nm}ov", name=f"{nm}ov")
                nc.vector.tensor_copy(ov[:, :], po[:, :])
                nc.sync.dma_start(out_t[:, :], ov[:, :])

    nc.compile()
    es.close()
    return pr


# --------------------------------------------------------------------------
# entry point
# --------------------------------------------------------------------------

_CACHE = {}


def make_in_maps(inputs, cfg, src_w, stt):
    x = np.asarray(inputs["x"], dtype=np.float32)
    shared = host_weights(inputs, cfg)
    in_maps = []
    for c in range(cfg.CORES):
        m = dict(shared)
        m["xT_slice"] = np.ascontiguousarray(
            x[c * cfg.NPC:(c + 1) * cfg.NPC].T.astype(BF))
        m["src_idx"] = np.ascontiguousarray(src_w[c])
        m["stt_tab"] = stt[c]
        in_maps.append(m)
    return in_maps


def run(inputs, cfg=CFG, trace=False):
    schedule, src_w, stt = host_prep(inputs["edge_index"], cfg)
    key = (cfg.N, cfg.E, tuple(schedule))
    if key not in _CACHE:
        _CACHE[key] = build_program(cfg, schedule)
    pr = _CACHE[key]
    in_maps = make_in_maps(inputs, cfg, src_w, stt)
    res = run_bass_kernel_spmd(pr.nc, in_maps, list(range(cfg.CORES)),
                               trace=trace)
    out = res.results[0]
    return (np.asarray(out["valence"], np.float32),
            np.asarray(out["arousal"], np.float32)), res


def kernel(**inputs):
    (val, aro), _ = run(inputs)
    return (val, aro)


# revision 103
# speedup vs baseline: 1.0343x; 1.0343x over previous
"""GCATopo (2-layer GTAT GNN) Trainium2 kernel, 8-way SPMD — v2.

Strategy (v2 redesign vs v1):
 - Node-major aggregation: per 128-edge tile ONE 512-wide matmul
   (lhsT=St one-hot, rhs=et2-weighted gathered features) accumulates
   [dst, 512] in a single PSUM bank; softmax denominators aggregate in a
   second small matmul. Normalization becomes per-partition scaling.
 - Per-edge dst logits come from a lookup matmul (lhsT=StT, rhs=local
   per-block dst-attn rows) instead of a 256B-per-edge DMA gather.
 - All per-edge elementwise work (logits, leaky-relu, exp, message
   weighting) is batched across a block's ~14 tiles with strided 3D/4D
   APs — a handful of DVE/Act instructions per block instead of ~15 per
   tile.
 - L2's topo output is discarded by the model, so L2 ships only
   [feat 512 | ta 4] and skips the SM stream entirely.
 - All matmul operands bf16 (4x PE rate vs f32); weights are host-folded
   (wl@attB etc.) and host-transposed; x arrives pre-transposed bf16.
 - Biases are folded forward into the next layer's constant rows, so
   drains are pure scaling.
 - L2 prep is fused into the L1 edge-phase block loop (PE prep matmuls
   overlap DVE/DMA edge work).
"""

from contextlib import ExitStack

import ml_dtypes
import numpy as np

import concourse.bacc as bacc
import concourse.tile as tile
from concourse import mybir
from concourse.masks import make_identity
from concourse.bass_utils import run_bass_kernel_spmd
from concourse.tile_rust import add_dep_helper

F32 = mybir.dt.float32
BF16 = mybir.dt.bfloat16
F8 = mybir.dt.float8e4
I16 = mybir.dt.int16
AF = mybir.ActivationFunctionType
OP = mybir.AluOpType

P = 128
BF = ml_dtypes.bfloat16


class Cfg:
    def __init__(self, N=20000, E=240000, FIN=576, HID=128, TOPO=15, H=4,
                 CORES=8, NEG=0.2):
        self.N, self.E, self.FIN, self.HID, self.TOPO, self.H = N, E, FIN, HID, TOPO, H
        self.CORES, self.NEG = CORES, NEG
        self.HC = H * HID                      # 512
        self.ROW = 768                         # gathered row: fp8 feat + pad
        self.RB = self.ROW // 2                # bf16 view width (384)
        self.NPC = N // CORES                  # nodes per core
        self.NBLK = (self.NPC + P - 1) // P    # dst blocks per core
        # aux slots within the BF16 VIEW of the row (bf16 element offsets;
        # feat occupies bf16-view [0:256))
        self.C_TOPO = 256                      # 256..270: topo (L1)
        self.C_ONE = 256 + TOPO                # 271: constant 1.0 (L1)
        self.C_AL = 272                        # 272..275: al (L1)
        self.C_TA = 276                        # 276..279: ta (L1)
        self.C_TA2 = 256                       # 256..259: ta (L2)


CFG = Cfg()
GT_MAX = 8  # max tiles (=128 idxs each) per gather call
GP_POOL_PCT = 25   # percent of Gp tiles offloaded DVE -> gpsimd (L1)
GP_POOL_PCT2 = 15  # same for L2 (gpsimd busier there)


def cdiv(a, b):
    return (a + b - 1) // b


def ktiles(F):
    return [(o, min(P, F - o)) for o in range(0, F, P)]


# --------------------------------------------------------------------------
# host-side graph preprocessing (pure indexing on edge_index)
# --------------------------------------------------------------------------

def host_prep(edge_index, cfg):
    N, CORES, NPC, NBLK = cfg.N, cfg.CORES, cfg.NPC, cfg.NBLK
    src = np.asarray(edge_index[0], dtype=np.int64)
    dst = np.asarray(edge_index[1], dtype=np.int64)
    loops = np.arange(N, dtype=np.int64)
    src = np.concatenate([src, loops])
    dst = np.concatenate([dst, loops])
    order = np.argsort(dst, kind="stable")
    s, d = src[order], dst[order]

    core_of = d // NPC
    blk_of = (d % NPC) // P
    counts = np.zeros((CORES, NBLK), dtype=np.int64)
    for c in range(CORES):
        m = core_of == c
        bb = blk_of[m]
        for b in range(NBLK):
            counts[c, b] = int((bb == b).sum())
    schedule = [max(1, cdiv(int(counts[:, b].max()), P)) for b in range(NBLK)]
    offs = np.concatenate([[0], np.cumsum(schedule)]).astype(np.int64)
    ttot = int(offs[-1])

    srcidx = np.zeros((CORES, ttot * P), dtype=np.int16)
    dstloc = np.full((CORES, ttot * P), -1.0, dtype=np.float32)
    near_cnt = np.zeros((CORES, NBLK), dtype=np.int64)
    percore = []
    for c in range(CORES):
        m = core_of == c
        sc, dc, bc = s[m], d[m], blk_of[m]
        plo, phi = (c // 2) * 2 * NPC, (c // 2 + 1) * 2 * NPC
        isnear = (sc >= plo) & (sc < phi)
        percore.append((sc, dc, bc, isnear))
        for b in range(NBLK):
            near_cnt[c, b] = int(((bc == b) & isnear).sum())
    tn = [min(int(near_cnt[:, b].min()) // P, schedule[b] - 1)
          for b in range(NBLK)]
    for c in range(CORES):
        sc, dc, bc, isnear = percore[c]
        for b in range(NBLK):
            mb = bc == b
            nb = int(mb.sum())
            ordr = np.argsort(~isnear[mb], kind="stable")  # near first
            base = int(offs[b]) * P
            srcidx[c, base:base + nb] = sc[mb][ordr].astype(np.int16)
            dstloc[c, base:base + nb] = (
                dc[mb][ordr] - (c * NPC + b * P)).astype(np.float32)
            pe = tn[b] * P      # near pads must hit a row the pair wrote
            if nb < pe:
                srcidx[c, base + nb:base + pe] = np.int16(c * NPC)

    # wrap for dma_gather: index i lives at [i % 16, i // 16]; the 16-row
    # block is replicated 8x along partitions (one stripe per gpsimd core)
    src_w = [np.tile(srcidx[c].reshape(-1, 16).T, (8, 1)).copy()
             for c in range(CORES)]
    # host-built one-hot selection tables, per tile [St | StT] (bf16 0/1):
    #   St[e, d] = (dstloc[e] == d), StT = St^T
    rng = np.arange(P, dtype=np.float32)
    stt = []
    for c in range(CORES):
        dl = dstloc[c].reshape(ttot, P)
        St = (dl[:, :, None] == rng[None, None, :])          # [t, e, d]
        tab = np.concatenate([St, St.transpose(0, 2, 1)], 2)  # [t, p, 256]
        stt.append(np.ascontiguousarray(
            tab.transpose(1, 0, 2).reshape(P, ttot * 2 * P).astype(BF)))
    return schedule, src_w, stt, tn


def host_weights(inputs, cfg):
    """All small-weight folding in f32 numpy, shipped as bf16."""
    H, C, TOPO, HC = cfg.H, cfg.HID, cfg.TOPO, cfg.HC
    f = lambda k: np.asarray(inputs[k], np.float32)

    def attB(att):  # [1,H,C] -> block-diag [H*C, H]
        out = np.zeros((H * C, H), np.float32)
        a = np.asarray(att, np.float32).reshape(H, C)
        for h in range(H):
            out[h * C:(h + 1) * C, h] = a[h]
        return out

    w = {}
    # topo extractor
    w["tw1"] = f("te_w1")                      # [576,128]
    w["tb1"] = f("te_b1").reshape(1, -1)
    w["tw2"] = f("te_w2")                      # [128,15]
    w["tb2"] = f("te_b2").reshape(1, -1)
    # layer 1
    aB1 = attB(inputs["l1_att"])
    w["wl1"] = f("l1_wl")                      # [576,512]
    w["bl1"] = f("l1_bl").reshape(1, -1)
    w["A1"] = np.concatenate([f("l1_wl") @ aB1, f("l1_wr") @ aB1], 1)  # [576,8]
    w["bA1"] = np.concatenate([f("l1_bl") @ aB1, f("l1_br") @ aB1]).reshape(1, -1)
    w["att2T1"] = f("l1_att2").reshape(H, TOPO).T      # [15,4]
    # layer 2 (input h1 = agg1_norm, l1_bias folded here)
    b1 = f("l1_bias")
    w["wl2"] = f("l2_wl")                      # [512,512]
    w["bl2"] = (b1 @ f("l2_wl") + f("l2_bl")).reshape(1, -1)
    w["att2T2"] = f("l2_att2").reshape(H, TOPO).T      # [15,4]
    # topo1 input to L2 = topo1_raw + l1_bias2; edge logit gets the const
    # twice (src+dst) -> fold 2*(b2@att2) into the dst-side rows only
    w["ta2c"] = (2.0 * (f("l1_bias2") @ w["att2T2"])).reshape(1, -1)   # [1,4]
    # heads (l2_bias folded into first-layer bias)
    b2f = f("l2_bias")
    for nm in ("v", "a"):
        w[f"{nm}w1"] = f(f"{nm}_w1")           # [512,128]
        w[f"{nm}b1"] = (f(f"{nm}_b1") + b2f @ f(f"{nm}_w1")).reshape(1, -1)
        w[f"{nm}w2"] = f(f"{nm}_w2")           # [128,1]
        w[f"{nm}b2"] = f(f"{nm}_b2").reshape(1, 1)
    # att2T2 flattened (h,j) row for the drain's ta2 reduce + const
    w["att2f"] = w["att2T2"].T.reshape(1, -1)  # [1,60] (h-major)
    return {k: v.astype(BF) for k, v in w.items()}


# --------------------------------------------------------------------------
# program builder
# --------------------------------------------------------------------------

class Prog:
    pass


def build_program(cfg, schedule, tn, debug=False):
    es = ExitStack()
    nc = bacc.Bacc("TRN2", target_bir_lowering=False, debug=False,
                   num_devices=cfg.CORES)
    pr = Prog()
    pr.nc = nc
    N, FIN, HID, TOPO, H, HC, ROW, NPC, NBLK = (
        cfg.N, cfg.FIN, cfg.HID, cfg.TOPO, cfg.H, cfg.HC, cfg.ROW, cfg.NPC,
        cfg.NBLK)
    TTOT = sum(schedule)
    W16 = TTOT * P // 16
    groups = [list(range(cfg.CORES))]
    blocks = ktiles(NPC)
    fkt = ktiles(FIN)
    ckt = ktiles(HC)
    offs = np.concatenate([[0], np.cumsum(schedule)]).astype(int)

    def din(name, shape, dtype=BF16):
        return nc.dram_tensor(name, list(shape), dtype, kind="ExternalInput")

    # ---- external inputs ----
    xT = din("xT_slice", (FIN, NPC))
    wnames = [("tw1", (FIN, HID)), ("tb1", (1, HID)), ("tw2", (HID, TOPO)),
              ("tb2", (1, TOPO)), ("wl1", (FIN, HC)), ("bl1", (1, HC)),
              ("A1", (FIN, 2 * H)), ("bA1", (1, 2 * H)), ("att2T1", (TOPO, H)),
              ("wl2", (HC, HC)), ("bl2", (1, HC)), ("att2T2", (TOPO, H)),
              ("ta2c", (1, H)), ("att2f", (1, H * TOPO)),
              ("vw1", (HC, HID)), ("vb1", (1, HID)), ("vw2", (HID, 1)),
              ("vb2", (1, 1)),
              ("aw1", (HC, HID)), ("ab1", (1, HID)), ("aw2", (HID, 1)),
              ("ab2", (1, 1))]
    W = {nm: din(nm, sh) for nm, sh in wnames}
    src_i = din("src_idx", (P, W16), I16)
    stt_i = din("stt_tab", (P, TTOT * 2 * P))

    # ---- outputs ----
    val_o = nc.dram_tensor("valence", [1, 1], F32, kind="ExternalOutput")
    aro_o = nc.dram_tensor("arousal", [1, 1], F32, kind="ExternalOutput")
    dbg = {}
    if debug:
        for nm, sh in [("dbg_h1", (P, HC)), ("dbg_tt", (P, TOPO + H)),
                       ("dbg_psm1", (P, 68)), ("dbg_h2", (P, HC)),
                       ("dbg_pool", (P, H)), ("dbg_aux", (P, 24)),
                       ("dbg_psm2", (P, H)), ("dbg_pd1", (P, 2 * H))]:
            dbg[nm] = nc.dram_tensor(nm, list(sh), F32, kind="ExternalOutput")

    # ---- internal DRAM ----
    ext_sl = [nc.dram_tensor(f"ext_slice{L}", [NPC, ROW], F8)
              for L in (1, 2)]
    ext_fl = [nc.dram_tensor(f"ext_full{L}", [N, ROW], F8,
                             addr_space="Shared") for L in (1, 2)]
    ext_pr = [nc.dram_tensor(f"ext_pair{L}", [N, ROW], F8,
                             addr_space="Shared") for L in (1, 2)]
    bar_io = [(nc.dram_tensor(f"bar_in{L}", [1, 1], F32),
               nc.dram_tensor(f"bar_out{L}", [2, 1], F32)) for L in (1, 2)]
    pgroups = [[2 * k, 2 * k + 1] for k in (0, 1, 2, 3)]
    pair_w = {1: [], 2: []}
    bar_inst = {}
    pool_in = nc.dram_tensor("pool_in", [1, HC], F32)
    pool_out = nc.dram_tensor("pool_out", [1, HC], F32, addr_space="Shared")

    with tile.TileContext(nc) as tc:
        # ================= static SBUF =================
        ident = nc.alloc_sbuf_tensor("ident", [P, P], F32).ap()
        make_identity(nc, ident)
        ones_row = nc.alloc_sbuf_tensor("ones_row", [1, NPC], BF16).ap()
        nc.gpsimd.memset(ones_row, 1.0)
        ones_col = nc.alloc_sbuf_tensor("ones_col", [P, 1], BF16).ap()
        nc.gpsimd.memset(ones_col, 1.0)
        eps_col = nc.alloc_sbuf_tensor("eps_col", [P, 1], F32).ap()
        nc.gpsimd.memset(eps_col, 1e-30)

        src_sb = nc.alloc_sbuf_tensor("src_sb", [P, W16], I16).ap()
        nc.sync.dma_start(src_sb, src_i[:, :])

        # resident activations / weights
        xT_sb = [nc.alloc_sbuf_tensor(f"xT{i}", [P, NPC], BF16).ap()
                 for i in range(len(fkt))]
        for i, (fo, fk) in enumerate(fkt):
            nc.sync.dma_start(xT_sb[i][:fk, :], xT[fo:fo + fk, :])
        hfmT = [nc.alloc_sbuf_tensor(f"hfmT{i}", [P, NPC], BF16).ap()
                for i in range(len(ckt))]
        topoT0 = nc.alloc_sbuf_tensor("topoT0", [TOPO, NPC], BF16).ap()
        datt1 = nc.alloc_sbuf_tensor("datt1", [P, NBLK * 2 * H], BF16).ap()
        datt2 = nc.alloc_sbuf_tensor("datt2", [P, NBLK * H], BF16).ap()
        nc.vector.memset(datt1, 0.0)   # rows past a partial block stay 0
        nc.vector.memset(datt2, 0.0)
        # near-pass partial aggregates (bf16 so the merge can be a matmul)
        partF = nc.alloc_sbuf_tensor("partF", [P, NBLK * HC], BF16).ap()
        partS = nc.alloc_sbuf_tensor("partS", [P, NBLK * (16 * H + H)],
                                     BF16).ap()
        pid = nc.partition_id()

        wsb = {}
        for nm, sh in wnames:
            if sh[0] <= P:
                wsb[nm] = nc.alloc_sbuf_tensor(f"w_{nm}", list(sh), BF16).ap()
                nc.sync.dma_start(wsb[nm], W[nm][:, :])
            else:  # k-tiled along the first (contraction) dim
                tiles = []
                for i, (fo, fk) in enumerate(ktiles(sh[0])):
                    t = nc.alloc_sbuf_tensor(f"w_{nm}{i}", [fk, sh[1]],
                                             BF16).ap()
                    nc.sync.dma_start(t, W[nm][fo:fo + fk, :])
                    tiles.append(t)
                wsb[nm] = tiles
        # att2f / ta2c broadcast to all partitions
        att2bc = nc.alloc_sbuf_tensor("att2bc", [P, H * TOPO], BF16).ap()
        nc.gpsimd.partition_broadcast(att2bc, wsb["att2f"][0:1, :])
        ta2cbc = nc.alloc_sbuf_tensor("ta2cbc", [P, H], BF16).ap()
        nc.gpsimd.partition_broadcast(ta2cbc, wsb["ta2c"][0:1, :])
        ident_bf = nc.alloc_sbuf_tensor("ident_bf", [P, P], BF16).ap()
        nc.vector.tensor_copy(ident_bf, ident)

        # ================= phase A: topo MLP + L1 prep =================
        with tc.tile_pool(name="ppA", bufs=1, space="PSUM") as ppA, \
             tc.tile_pool(name="ppA2", bufs=2, space="PSUM") as ppA2, \
             tc.tile_pool(name="cpA", bufs=3) as cpA, \
             tc.tile_pool(name="spA", bufs=2) as spA:
            # --- topo extractor MLP (feat-major: out rows = hid/topo) ---
            NG = 512
            for go in range(0, NPC, NG):
                gs = min(NG, NPC - go)
                ph = ppA.tile([P, NG], F32, tag="ph", name="ph", space="PSUM")
                for i, (fo, fk) in enumerate(fkt):
                    nc.tensor.matmul(ph[:, :gs], lhsT=wsb["tw1"][i][:fk, :],
                                     rhs=xT_sb[i][:fk, go:go + gs],
                                     start=i == 0, stop=False,
                                     skip_group_check=True)
                nc.tensor.matmul(ph[:, :gs], lhsT=wsb["tb1"][:, :],
                                 rhs=ones_row[:, go:go + gs], start=False,
                                 stop=True, skip_group_check=True)
                t_hid = spA.tile([P, NG], BF16, tag="t_hid", name="t_hid")
                nc.scalar.activation(t_hid[:, :gs], ph[:, :gs], AF.Relu)
                pt = ppA.tile([TOPO, NG], F32, tag="pt", name="pt", space="PSUM")
                nc.tensor.matmul(pt[:, :gs], lhsT=wsb["tw2"][:, :],
                                 rhs=t_hid[:, :gs], start=True, stop=False,
                                 skip_group_check=True)
                nc.tensor.matmul(pt[:, :gs], lhsT=wsb["tb2"][:, :],
                                 rhs=ones_row[:, go:go + gs], start=False,
                                 stop=True, skip_group_check=True)
                nc.vector.tensor_copy(topoT0[:, go:go + gs], pt[:, :gs])

            # --- L1 prep per block ---
            for bi, (bo, bs) in enumerate(blocks):
                pm = ppA2.tile([P, HC], F32, tag="pm", name="pm", space="PSUM")
                pa = ppA.tile([P, 2 * H], F32, tag="pa", name="pa", space="PSUM")
                for i, (fo, fk) in enumerate(fkt):
                    nc.tensor.matmul(pm[:bs, :], lhsT=xT_sb[i][:fk, bo:bo + bs],
                                     rhs=wsb["wl1"][i][:fk, :],
                                     start=i == 0, stop=False,
                                     skip_group_check=True)
                    nc.tensor.matmul(pa[:bs, :], lhsT=xT_sb[i][:fk, bo:bo + bs],
                                     rhs=wsb["A1"][i][:fk, :],
                                     start=i == 0, stop=False,
                                     skip_group_check=True)
                nc.tensor.matmul(pm[:bs, :], lhsT=ones_row[:, bo:bo + bs],
                                 rhs=wsb["bl1"][:, :], start=False, stop=True,
                                 skip_group_check=True)
                nc.tensor.matmul(pa[:bs, :], lhsT=ones_row[:, bo:bo + bs],
                                 rhs=wsb["bA1"][:, :], start=False, stop=True,
                                 skip_group_check=True)
                pta = ppA.tile([P, H], F32, tag="pta", name="pta", space="PSUM")
                nc.tensor.matmul(pta[:bs, :], lhsT=topoT0[:, bo:bo + bs],
                                 rhs=wsb["att2T1"][:, :], start=True,
                                 stop=True, skip_group_check=True)
                ptt = ppA.tile([P, TOPO], BF16, tag="ptt", name="ptt",
                               space="PSUM")
                nc.tensor.transpose(ptt[:bs, :TOPO],
                                    topoT0[:, bo:bo + bs],
                                    ident_bf[:TOPO, :TOPO])
                ext = cpA.tile([P, ROW], F8, tag="ext", name="ext")
                extb = ext[:, :].bitcast(BF16)
                nc.scalar.copy(ext[:bs, 0:HC], pm[:bs, :])
                nc.scalar.copy(extb[:bs, cfg.C_TOPO:cfg.C_TOPO + TOPO],
                               ptt[:bs, :TOPO])
                nc.vector.memset(extb[:bs, cfg.C_ONE:cfg.C_ONE + 1], 1.0)
                nc.scalar.copy(extb[:bs, cfg.C_AL:cfg.C_AL + H], pa[:bs, 0:H])
                nc.scalar.copy(extb[:bs, cfg.C_TA:cfg.C_TA + H], pta[:bs, :])
                nc.sync.dma_start(ext_sl[0][bo:bo + bs, :], ext[:bs, :])
                wpr = nc.sync.dma_start(
                    ext_pr[0][:, :].rearrange("(c n) r -> c n r",
                                              c=cfg.CORES)[pid][bo:bo + bs, :],
                    ext[:bs, :])
                pair_w[1].append(wpr.ins)
                if debug and bi == 0:
                    da = cpA.tile([P, 24], F32, tag="dbga", name="dbga")
                    nc.vector.tensor_copy(da[:, :],
                                          extb[:, cfg.C_TOPO:cfg.C_TOPO + 24])
                    nc.sync.dma_start(dbg["dbg_aux"][:, :], da[:, :])
                # dst-side rows: [ar | ta]
                nc.vector.tensor_copy(datt1[:bs, bi * 2 * H:bi * 2 * H + H],
                                      pa[:bs, H:2 * H])
                nc.vector.tensor_copy(
                    datt1[:bs, bi * 2 * H + H:(bi + 1) * 2 * H], pta[:bs, :])
            pass  # collectives for L1 are emitted inside the phase-B scope

        # ================= edge phase (shared emitter) =================
        TMAX = max(schedule)

        def emit_collectives(L):
            # cheap pair barrier, PINNED before the AllGather so the
            # near pass (which waits only on the barrier) overlaps the AG
            bar = nc.gpsimd.collective_compute(
                "AllGather", OP.bypass, replica_groups=pgroups,
                ins=[bar_io[L - 1][0][:, :]], outs=[bar_io[L - 1][1][:, :]])
            for w in pair_w[L]:
                add_dep_helper(bar.ins, w, reason="pair barrier")
            bar_inst[L] = bar
            ag = nc.gpsimd.collective_compute(
                "AllGather", OP.bypass, replica_groups=groups,
                ins=[ext_sl[L - 1][:, :]], outs=[ext_fl[L - 1][:, :]])
            add_dep_helper(ag.ins, bar.ins, reason="pin barrier before AG")

        SMW1 = 16 * H + H

        def emit_tiles(L, gp, sp, pp, pp2, bi, ta, te_, src_t, dep_i, mrg):
            """Tiles [ta, te_) of block bi -> (pf, psm) psums."""
            AUXW = 2 * H if L == 1 else H      # lg width per tile
            AUXO = cfg.C_AL if L == 1 else cfg.C_TA2
            base = int(offs[bi]) + ta
            nt = te_ - ta
            SMW = SMW1 if L == 1 else H
            # ---- gathers ----
            G = gp.tile([P, TMAX * ROW], F8, tag="G", name="G")
            for go in range(0, nt, GT_MAX):
                gn = min(GT_MAX, nt - go)
                c0 = (base + go) * 8
                gi = nc.gpsimd.dma_gather(
                    G[:, go * ROW:(go + gn) * ROW].rearrange(
                        "p (t e) -> p t e", e=ROW),
                    src_t[:, :], src_sb[:, c0:c0 + 8 * gn],
                    num_idxs=P * gn, num_idxs_reg=P * gn, elem_size=ROW,
                    queue_num=0)
                if dep_i is not None:
                    add_dep_helper(gi.ins, dep_i.ins,
                                   reason="near gather after pair barrier")
            # ---- St / StT (host-built one-hot tables) ----
            stt = sp.tile([P, TMAX * 2 * P], BF16, tag="stt", name="stt")
            nc.sync.dma_start(stt[:, 0:nt * 2 * P],
                              stt_i[:, base * 2 * P:(base + nt) * 2 * P])

            def St(t):
                return stt[:, t * 2 * P:t * 2 * P + P]

            def StT(t):
                return stt[:, t * 2 * P + P:(t + 1) * 2 * P]
            # ---- dst-logit lookup ----
            pD = pp.tile([P, TMAX * AUXW], F32, tag="pD", name="pD",
                         space="PSUM")
            dsl = (datt1[:, bi * 2 * H:(bi + 1) * 2 * H] if L == 1
                   else datt2[:, bi * H:(bi + 1) * H])
            for t in range(nt):
                nc.tensor.matmul(pD[:, t * AUXW:(t + 1) * AUXW],
                                 lhsT=StT(t), rhs=dsl, start=True,
                                 stop=True, skip_group_check=True)
            # ---- batched logits ----
            Gb = G[:, 0:nt * ROW].bitcast(BF16).rearrange(
                "p (t e) -> p t e", e=cfg.RB)
            lg = sp.tile([P, TMAX * AUXW], F32, tag="lg", name="lg")
            nc.vector.tensor_tensor(
                lg[:, 0:nt * AUXW].rearrange("p (t c) -> p t c", c=AUXW),
                Gb[:, :, AUXO:AUXO + AUXW],
                pD[:, 0:nt * AUXW].rearrange("p (t c) -> p t c", c=AUXW),
                OP.add)
            lr = sp.tile([P, TMAX * AUXW], F32, tag="lr", name="lr")
            nc.vector.scalar_tensor_tensor(
                lr[:, 0:nt * AUXW], lg[:, 0:nt * AUXW], cfg.NEG,
                lg[:, 0:nt * AUXW], OP.mult, OP.max)
            et = sp.tile([P, TMAX * AUXW], BF16, tag="et", name="et")
            nc.scalar.activation(et[:, 0:nt * AUXW], lr[:, 0:nt * AUXW],
                                 AF.Exp)
            etv = et[:, 0:nt * AUXW].rearrange("p (t c) -> p t c", c=AUXW)
            # ---- weighted messages (split DVE / gpsimd) ----
            Gp = gp.tile([P, TMAX * HC], BF16, tag="Gp", name="Gp")
            e2off = H if L == 1 else 0
            Gf = G[:, 0:nt * ROW].rearrange("p (t e) -> p t e", e=ROW)
            pct = GP_POOL_PCT if L == 1 else GP_POOL_PCT2
            ks = (nt * pct + 99) // 100      # first ks tiles on Pool

            def gp_op(eng, t0, t1):
                if t1 <= t0:
                    return
                eng.tensor_tensor(
                    Gp[:, t0 * HC:t1 * HC].rearrange(
                        "p (t h c) -> p t h c", h=H, c=HID),
                    Gf[:, t0:t1, 0:HC].rearrange(
                        "p t (h c) -> p t h c", c=HID),
                    etv[:, t0:t1, e2off:e2off + H].unsqueeze(
                        3).to_broadcast((P, t1 - t0, H, HID)),
                    OP.mult)
            for t0 in range(0, ks, 3):        # gpsimd in 3-tile chunks
                gp_op(nc.gpsimd, t0, min(t0 + 3, ks))
            for t0 in range(ks, nt, 4):       # DVE in 4-tile chunks
                gp_op(nc.vector, t0, min(t0 + 4, nt))
            if L == 1:
                SMe = sp.tile([P, TMAX * SMW], BF16, tag="SMe", name="SMe")
                SMv = SMe[:, 0:nt * SMW].rearrange("p (t c) -> p t c", c=SMW)
                nc.vector.tensor_tensor(
                    SMv[:, :, 0:16 * H].rearrange(
                        "p t (h j) -> p t h j", j=16),
                    Gb[:, :, cfg.C_TOPO:cfg.C_TOPO + 16].unsqueeze(
                        2).to_broadcast((P, nt, H, 16)),
                    etv[:, :, 0:H].unsqueeze(3).to_broadcast(
                        (P, nt, H, 16)),
                    OP.mult)
                nc.scalar.copy(SMv[:, :, 16 * H:SMW], etv[:, :, H:2 * H])
            else:
                SMe = et
            # ---- aggregation matmuls (mrg: preload stashed partials) ----
            pf = pp2.tile([P, HC], F32, tag="pf", name="pf", space="PSUM")
            psm = pp2.tile([P, SMW], F32, tag="psm", name="psm",
                           space="PSUM")
            if mrg:
                nc.tensor.matmul(pf[:, :], lhsT=ident_bf,
                                 rhs=partF[:, bi * HC:(bi + 1) * HC],
                                 start=True, stop=False,
                                 skip_group_check=True)
                nc.tensor.matmul(psm[:, :], lhsT=ident_bf,
                                 rhs=partS[:, bi * SMW1:bi * SMW1 + SMW],
                                 start=True, stop=False,
                                 skip_group_check=True)
            for t in range(nt):
                st0 = (t == 0) and not mrg
                sp1 = t == nt - 1
                nc.tensor.matmul(pf[:, :], lhsT=St(t),
                                 rhs=Gp[:, t * HC:(t + 1) * HC],
                                 start=st0, stop=sp1, skip_group_check=True)
                nc.tensor.matmul(psm[:, :], lhsT=St(t),
                                 rhs=SMe[:, t * SMW:(t + 1) * SMW],
                                 start=st0, stop=sp1, skip_group_check=True)
            return pf, psm

        def emit_edge(L, gp, sp, pp, pp2):
            SMW = SMW1 if L == 1 else H
            # near pass: pair-local tiles, overlapping the AllGather
            for bi in range(len(blocks)):
                if tn[bi] <= 0:
                    continue
                pf, psm = emit_tiles(L, gp, sp, pp, pp2, bi, 0, tn[bi],
                                     ext_pr[L - 1], bar_inst[L], False)
                nc.scalar.copy(partF[:, bi * HC:(bi + 1) * HC], pf[:, :])
                nc.scalar.copy(partS[:, bi * SMW1:bi * SMW1 + SMW],
                               psm[:, :])
            # far pass after the AllGather, merging the stash
            for bi, (bo, bs) in enumerate(blocks):
                pf, psm = emit_tiles(L, gp, sp, pp, pp2, bi, tn[bi],
                                     schedule[bi], ext_fl[L - 1], None,
                                     tn[bi] > 0)
                if L == 1:
                    drain1(bi, bo, bs, pf, psm, sp, pp, pp2)
                else:
                    drain2(bi, bo, bs, pf, psm, sp, pp)

        # ---- L1 drain + fused L2 prep ----
        def drain1(bi, bo, bs, pf, psm, sp, pp, pp2):
            # rec2 = 1/sum(e2), rec1' = 1/(H*sum(e1))
            den = sp.tile([P, 2 * H], F32, tag="den", name="den")
            nc.vector.tensor_scalar(
                den[:, 0:H].unsqueeze(2),
                psm[:, 0:16 * H].rearrange("p (h j) -> p h j", j=16)[
                    :, :, 15:16],
                float(H), eps_col[:, 0:1], OP.mult, OP.max)
            nc.vector.tensor_tensor(den[:, H:2 * H], psm[:, 16 * H:16 * H + H],
                                    eps_col[:, 0:1].to_broadcast((P, H)),
                                    OP.max)
            rec = sp.tile([P, 2 * H], F32, tag="rec", name="rec")
            nc.vector.reciprocal(rec[:, :], den[:, :])
            # h1 = agg_feat * rec2 (node-major, bf16; per-head scale on Act)
            h1 = sp.tile([P, HC], BF16, tag="h1", name="h1")
            for h in range(H):
                nc.scalar.activation(h1[:, h * HID:(h + 1) * HID],
                                     pf[:, h * HID:(h + 1) * HID], AF.Copy,
                                     scale=rec[:, H + h:H + h + 1])
            # topo1_raw = sum_h agg_topo_h * rec1'   [d, 15]
            tp = sp.tile([P, TOPO * H], F32, tag="tp", name="tp")
            nc.vector.tensor_tensor(
                tp[:, :].rearrange("p (j h) -> p j h", h=H),
                psm[:, 0:16 * H].rearrange("p (h j) -> p h j", j=16)[
                    :, :, 0:TOPO].transpose([0, 2, 1]),
                rec[:, 0:H].unsqueeze(1).to_broadcast((P, TOPO, H)),
                OP.mult)
            t1 = sp.tile([P, TOPO], F32, tag="t1", name="t1")
            nc.vector.tensor_reduce(
                t1[:, :], tp[:, :].rearrange("p (j h) -> p j h", h=H),
                mybir.AxisListType.X, OP.add)
            # ta2 = topo1_raw @ att2T2 (per-node, via DVE reduce)
            tq = sp.tile([P, H * TOPO], F32, tag="tq", name="tq")
            nc.vector.tensor_tensor(
                tq[:, :].rearrange("p (h j) -> p h j", j=TOPO),
                t1[:, :].unsqueeze(1).to_broadcast((P, H, TOPO)),
                att2bc[:, :].rearrange("p (h j) -> p h j", j=TOPO),
                OP.mult)
            ta2 = sp.tile([P, H], F32, tag="ta2", name="ta2")
            nc.vector.tensor_reduce(
                ta2[:, :], tq[:, :].rearrange("p (h j) -> p h j", j=TOPO),
                mybir.AxisListType.X, OP.add)
            # dst rows for L2: ta2 + 2*(b2@att2)
            nc.vector.tensor_tensor(datt2[:bs, bi * H:(bi + 1) * H],
                                    ta2[:bs, :],
                                    ta2cbc[:bs, :], OP.add)
            if debug and bi == 0:
                dh = sp.tile([P, HC], F32, tag="dbgh", name="dbgh")
                nc.vector.tensor_copy(dh[:, :], h1[:, :])
                nc.sync.dma_start(dbg["dbg_h1"][:, :], dh[:, :])
                dtt = sp.tile([P, TOPO + H], F32, tag="dbgt", name="dbgt")
                nc.vector.tensor_copy(dtt[:, 0:TOPO], t1[:, :])
                nc.vector.tensor_copy(dtt[:, TOPO:TOPO + H], ta2[:, :])
                nc.sync.dma_start(dbg["dbg_tt"][:, :], dtt[:, :])
            # transpose h1 -> hfmT tiles
            for ci, (co, ck) in enumerate(ckt):
                ptr = pp.tile([P, P], BF16, tag="ptr", name="ptr",
                              space="PSUM")
                nc.tensor.transpose(ptr[:ck, :bs], h1[:bs, co:co + ck],
                                    ident_bf[:bs, :bs])
                nc.scalar.copy(hfmT[ci][:ck, bo:bo + bs], ptr[:ck, :bs])
            # ---- fused L2 prep for this block ----
            pm2 = pp2.tile([P, HC], F32, tag="pm2", name="pm2", space="PSUM")
            for ci, (co, ck) in enumerate(ckt):
                nc.tensor.matmul(pm2[:bs, :], lhsT=hfmT[ci][:ck, bo:bo + bs],
                                 rhs=wsb["wl2"][ci][:ck, :],
                                 start=ci == 0, stop=False,
                                 skip_group_check=True)
            nc.tensor.matmul(pm2[:bs, :], lhsT=ones_row[:, bo:bo + bs],
                             rhs=wsb["bl2"][:, :], start=False, stop=True,
                             skip_group_check=True)
            ext = sp.tile([P, ROW], F8, tag="ext2", name="ext2")
            nc.scalar.copy(ext[:bs, 0:HC], pm2[:bs, :])
            nc.scalar.copy(ext[:, :].bitcast(BF16)[
                :bs, cfg.C_TA2:cfg.C_TA2 + H], ta2[:bs, :])
            nc.sync.dma_start(ext_sl[1][bo:bo + bs, :], ext[:bs, :])
            wpr = nc.sync.dma_start(
                ext_pr[1][:, :].rearrange("(c n) r -> c n r",
                                          c=cfg.CORES)[pid][bo:bo + bs, :],
                ext[:bs, :])
            pair_w[2].append(wpr.ins)

        # ---- L2 drain: normalize + pooled partial ----
        def drain2(bi, bo, bs, pf, psm, sp, pp):
            den = sp.tile([P, H], F32, tag="den2", name="den2")
            nc.vector.tensor_tensor(den[:, :], psm[:, 0:H],
                                    eps_col[:, 0:1].to_broadcast((P, H)),
                                    OP.max)
            rec = sp.tile([P, H], F32, tag="rec2", name="rec2")
            nc.vector.reciprocal(rec[:, :], den[:, :])
            h2 = sp.tile([P, HC], BF16, tag="h2", name="h2")
            for h in range(H):
                nc.scalar.activation(h2[:, h * HID:(h + 1) * HID],
                                     pf[:, h * HID:(h + 1) * HID], AF.Copy,
                                     scale=rec[:, h:h + 1])
            if debug and bi == 0:
                dh = sp.tile([P, HC], F32, tag="dbgh2", name="dbgh2")
                nc.vector.tensor_copy(dh[:, :], h2[:, :])
                nc.sync.dma_start(dbg["dbg_h2"][:, :], dh[:, :])
            nc.tensor.matmul(pr.pool_ps[:, :], lhsT=ones_col[:bs, 0:1],
                             rhs=h2[:bs, :], start=bi == 0,
                             stop=bi == len(blocks) - 1,
                             skip_group_check=True)

        # ================= phase B/C: L1 edges (+L2 prep) =================
        # PSUM banks: (pf+psm) 2x2 + pm2 2 + pD/ptr 1 each = 8 of 8
        with tc.tile_pool(name="gpB", bufs=2) as gpB, \
             tc.tile_pool(name="spB", bufs=3) as spB, \
             tc.tile_pool(name="ppB", bufs=1, space="PSUM") as ppB, \
             tc.tile_pool(name="ppB2", bufs=2, space="PSUM") as ppB2:
            emit_collectives(1)
            emit_edge(1, gpB, spB, ppB, ppB2)

        # ================= phase D: L2 edges =================
        with tc.tile_pool(name="gpD", bufs=2) as gpD, \
             tc.tile_pool(name="spD", bufs=3) as spD, \
             tc.tile_pool(name="ppD", bufs=1, space="PSUM") as ppD, \
             tc.tile_pool(name="ppD2", bufs=2, space="PSUM") as ppD2, \
             tc.tile_pool(name="plD", bufs=1, space="PSUM") as plD:
            pr.pool_ps = plD.tile([1, HC], F32, tag="pool", name="pool",
                                  space="PSUM", bufs=1)
            emit_collectives(2)
            emit_edge(2, gpD, spD, ppD, ppD2)

            # ---- pool + heads ----
            pooled = spD.tile([1, HC], F32, tag="pooled", name="pooled")
            nc.vector.tensor_copy(pooled[:, :], pr.pool_ps[:, :])
            nc.sync.dma_start(pool_in[:, :], pooled[:, :])
            nc.gpsimd.collective_compute(
                "AllReduce", OP.add, replica_groups=groups,
                ins=[pool_in[:, :]], outs=[pool_out[:, :]])
            # load back column-major: pmean_cols[c, h] = pool_out[h*HID+c]
            pooled2 = spD.tile([P, H], F32, tag="pooled2", name="pooled2")
            with nc.allow_non_contiguous_dma("pool row -> col-major reload"):
                nc.sync.dma_start(
                    pooled2[:, :],
                    pool_out[:, :].rearrange("o (h c) -> (o c) h", c=HID))
            if debug:
                nc.sync.dma_start(dbg["dbg_pool"][:, :], pooled2[:, :])
            pmean = spD.tile([P, H], BF16, tag="pmean", name="pmean")
            nc.vector.tensor_scalar(pmean[:, :], pooled2[:, :], 1.0 / N,
                                    None, OP.mult)
            for nm, out_t in (("v", val_o), ("a", aro_o)):
                pm = ppD.tile([P, 1], F32, tag="mlp", name="mlp", space="PSUM")
                for ki in range(H):
                    nc.tensor.matmul(pm[:, :], lhsT=wsb[f"{nm}w1"][ki][:, :],
                                     rhs=pmean[:, ki:ki + 1], start=ki == 0,
                                     stop=False, skip_group_check=True)
                nc.tensor.matmul(pm[:, :], lhsT=wsb[f"{nm}b1"][:, :],
                                 rhs=ones_col[0:1, :], start=False, stop=True,
                                 skip_group_check=True)
                hv = spD.tile([P, 1], BF16, tag=f"{nm}hv", name=f"{nm}hv")
                nc.scalar.activation(hv[:, :], pm[:, :], AF.Relu)
                po = ppD.tile([1, 1], F32, tag="mlpo", name="mlpo",
                              space="PSUM")
                nc.tensor.matmul(po[:, :], lhsT=hv[:, :],
                                 rhs=wsb[f"{nm}w2"][:, :], start=True,
                                 stop=False, skip_group_check=True)
                nc.tensor.matmul(po[:, :], lhsT=wsb[f"{nm}b2"][:, :],
                                 rhs=ones_col[0:1, :], start=False, stop=True,
                                 skip_group_check=True)
                ov = spD.tile([1, 1], F32, tag=f"{nm}ov", name=f"{nm}ov")
                nc.vector.tensor_copy(ov[:, :], po[:, :])
                nc.sync.dma_start(out_t[:, :], ov[:, :])

    nc.compile()
    es.close()
    return pr


# --------------------------------------------------------------------------
# entry point
# --------------------------------------------------------------------------

_CACHE = {}


def make_in_maps(inputs, cfg, src_w, stt):
    x = np.asarray(inputs["x"], dtype=np.float32)
    shared = host_weights(inputs, cfg)
    in_maps = []
    for c in range(cfg.CORES):
        m = dict(shared)
        m["xT_slice"] = np.ascontiguousarray(
            x[c * cfg.NPC:(c + 1) * cfg.NPC].T.astype(BF))
        m["src_idx"] = np.ascontiguousarray(src_w[c])
        m["stt_tab"] = stt[c]
        in_maps.append(m)
    return in_maps


def run(inputs, cfg=CFG, trace=False):
    schedule, src_w, stt, tn = host_prep(inputs["edge_index"], cfg)
    key = (cfg.N, cfg.E, tuple(schedule), tuple(tn))
    if key not in _CACHE:
        _CACHE[key] = build_program(cfg, schedule, tn)
    pr = _CACHE[key]
    in_maps = make_in_maps(inputs, cfg, src_w, stt)
    res = run_bass_kernel_spmd(pr.nc, in_maps, list(range(cfg.CORES)),
                               trace=trace)
    out = res.results[0]
    return (np.asarray(out["valence"], np.float32),
            np.asarray(out["arousal"], np.float32)), res


def kernel(**inputs):
    (val, aro), _ = run(inputs)
    return (val, aro)


# revision 104
# speedup vs baseline: 1.0529x; 1.0180x over previous
"""GCATopo (2-layer GTAT GNN) Trainium2 kernel, 8-way SPMD — v2.

Strategy (v2 redesign vs v1):
 - Node-major aggregation: per 128-edge tile ONE 512-wide matmul
   (lhsT=St one-hot, rhs=et2-weighted gathered features) accumulates
   [dst, 512] in a single PSUM bank; softmax denominators aggregate in a
   second small matmul. Normalization becomes per-partition scaling.
 - Per-edge dst logits come from a lookup matmul (lhsT=StT, rhs=local
   per-block dst-attn rows) instead of a 256B-per-edge DMA gather.
 - All per-edge elementwise work (logits, leaky-relu, exp, message
   weighting) is batched across a block's ~14 tiles with strided 3D/4D
   APs — a handful of DVE/Act instructions per block instead of ~15 per
   tile.
 - L2's topo output is discarded by the model, so L2 ships only
   [feat 512 | ta 4] and skips the SM stream entirely.
 - All matmul operands bf16 (4x PE rate vs f32); weights are host-folded
   (wl@attB etc.) and host-transposed; x arrives pre-transposed bf16.
 - Biases are folded forward into the next layer's constant rows, so
   drains are pure scaling.
 - L2 prep is fused into the L1 edge-phase block loop (PE prep matmuls
   overlap DVE/DMA edge work).
"""

from contextlib import ExitStack

import ml_dtypes
import numpy as np

import concourse.bacc as bacc
import concourse.tile as tile
from concourse import mybir
from concourse.masks import make_identity
from concourse.bass_utils import run_bass_kernel_spmd
from concourse.tile_rust import add_dep_helper

F32 = mybir.dt.float32
BF16 = mybir.dt.bfloat16
F8 = mybir.dt.float8e4
I16 = mybir.dt.int16
AF = mybir.ActivationFunctionType
OP = mybir.AluOpType

P = 128
BF = ml_dtypes.bfloat16


class Cfg:
    def __init__(self, N=20000, E=240000, FIN=576, HID=128, TOPO=15, H=4,
                 CORES=8, NEG=0.2):
        self.N, self.E, self.FIN, self.HID, self.TOPO, self.H = N, E, FIN, HID, TOPO, H
        self.CORES, self.NEG = CORES, NEG
        self.HC = H * HID                      # 512
        self.ROW = 768                         # gathered row: fp8 feat + pad
        self.RB = self.ROW // 2                # bf16 view width (384)
        self.NPC = N // CORES                  # nodes per core
        self.NBLK = (self.NPC + P - 1) // P    # dst blocks per core
        # aux slots within the BF16 VIEW of the row (bf16 element offsets;
        # feat occupies bf16-view [0:256))
        self.C_TOPO = 256                      # 256..270: topo (L1)
        self.C_ONE = 256 + TOPO                # 271: constant 1.0 (L1)
        self.C_AL = 272                        # 272..275: al (L1)
        self.C_TA = 276                        # 276..279: ta (L1)
        self.C_TA2 = 256                       # 256..259: ta (L2)


CFG = Cfg()
GT_MAX = 8  # max tiles (=128 idxs each) per gather call
GP_POOL_PCT = 25   # percent of Gp tiles offloaded DVE -> gpsimd (L1)
GP_POOL_PCT2 = 15  # same for L2 (gpsimd busier there)


def cdiv(a, b):
    return (a + b - 1) // b


def ktiles(F):
    return [(o, min(P, F - o)) for o in range(0, F, P)]


# --------------------------------------------------------------------------
# host-side graph preprocessing (pure indexing on edge_index)
# --------------------------------------------------------------------------

def host_prep(edge_index, cfg):
    N, CORES, NPC, NBLK = cfg.N, cfg.CORES, cfg.NPC, cfg.NBLK
    src = np.asarray(edge_index[0], dtype=np.int64)
    dst = np.asarray(edge_index[1], dtype=np.int64)
    loops = np.arange(N, dtype=np.int64)
    src = np.concatenate([src, loops])
    dst = np.concatenate([dst, loops])
    order = np.argsort(dst, kind="stable")
    s, d = src[order], dst[order]

    core_of = d // NPC
    blk_of = (d % NPC) // P
    near_cnt = np.zeros((CORES, NBLK), dtype=np.int64)
    far_cnt = np.zeros((CORES, NBLK), dtype=np.int64)
    percore = []
    for c in range(CORES):
        m = core_of == c
        sc, dc, bc = s[m], d[m], blk_of[m]
        plo, phi = (c // 2) * 2 * NPC, (c // 2 + 1) * 2 * NPC
        isnear = (sc >= plo) & (sc < phi)
        percore.append((sc, dc, bc, isnear))
        for b in range(NBLK):
            near_cnt[c, b] = int(((bc == b) & isnear).sum())
            far_cnt[c, b] = int(((bc == b) & ~isnear).sum())
    # tn = ceil over cores (cores with fewer near edges pad with dummies
    # that point at an own row with dl=-1, contributing nothing)
    tn = [cdiv(int(near_cnt[:, b].max()), P) for b in range(NBLK)]
    schedule = [tn[b] + max(1, cdiv(int(far_cnt[:, b].max()), P))
                for b in range(NBLK)]
    offs = np.concatenate([[0], np.cumsum(schedule)]).astype(np.int64)
    ttot = int(offs[-1])

    srcidx = np.zeros((CORES, ttot * P), dtype=np.int16)
    dstloc = np.full((CORES, ttot * P), -1.0, dtype=np.float32)
    for c in range(CORES):
        sc, dc, bc, isnear = percore[c]
        for b in range(NBLK):
            base = int(offs[b]) * P
            nm_ = (bc == b) & isnear
            fm_ = (bc == b) & ~isnear
            nn, nf = int(nm_.sum()), int(fm_.sum())
            srcidx[c, base:base + nn] = sc[nm_].astype(np.int16)
            dstloc[c, base:base + nn] = (
                dc[nm_] - (c * NPC + b * P)).astype(np.float32)
            srcidx[c, base + nn:base + tn[b] * P] = np.int16(c * NPC)
            fs = base + tn[b] * P
            srcidx[c, fs:fs + nf] = sc[fm_].astype(np.int16)
            dstloc[c, fs:fs + nf] = (
                dc[fm_] - (c * NPC + b * P)).astype(np.float32)

    # wrap for dma_gather: index i lives at [i % 16, i // 16]; the 16-row
    # block is replicated 8x along partitions (one stripe per gpsimd core)
    src_w = [np.tile(srcidx[c].reshape(-1, 16).T, (8, 1)).copy()
             for c in range(CORES)]
    # host-built one-hot selection tables, per tile [St | StT] (bf16 0/1):
    #   St[e, d] = (dstloc[e] == d), StT = St^T
    rng = np.arange(P, dtype=np.float32)
    stt = []
    for c in range(CORES):
        dl = dstloc[c].reshape(ttot, P)
        St = (dl[:, :, None] == rng[None, None, :])          # [t, e, d]
        tab = np.concatenate([St, St.transpose(0, 2, 1)], 2)  # [t, p, 256]
        stt.append(np.ascontiguousarray(
            tab.transpose(1, 0, 2).reshape(P, ttot * 2 * P).astype(BF)))
    return schedule, src_w, stt, tn


def host_weights(inputs, cfg):
    """All small-weight folding in f32 numpy, shipped as bf16."""
    H, C, TOPO, HC = cfg.H, cfg.HID, cfg.TOPO, cfg.HC
    f = lambda k: np.asarray(inputs[k], np.float32)

    def attB(att):  # [1,H,C] -> block-diag [H*C, H]
        out = np.zeros((H * C, H), np.float32)
        a = np.asarray(att, np.float32).reshape(H, C)
        for h in range(H):
            out[h * C:(h + 1) * C, h] = a[h]
        return out

    w = {}
    # topo extractor
    w["tw1"] = f("te_w1")                      # [576,128]
    w["tb1"] = f("te_b1").reshape(1, -1)
    w["tw2"] = f("te_w2")                      # [128,15]
    w["tb2"] = f("te_b2").reshape(1, -1)
    # layer 1
    aB1 = attB(inputs["l1_att"])
    w["wl1"] = f("l1_wl")                      # [576,512]
    w["bl1"] = f("l1_bl").reshape(1, -1)
    w["A1"] = np.concatenate([f("l1_wl") @ aB1, f("l1_wr") @ aB1], 1)  # [576,8]
    w["bA1"] = np.concatenate([f("l1_bl") @ aB1, f("l1_br") @ aB1]).reshape(1, -1)
    w["att2T1"] = f("l1_att2").reshape(H, TOPO).T      # [15,4]
    # layer 2 (input h1 = agg1_norm, l1_bias folded here)
    b1 = f("l1_bias")
    w["wl2"] = f("l2_wl")                      # [512,512]
    w["bl2"] = (b1 @ f("l2_wl") + f("l2_bl")).reshape(1, -1)
    w["att2T2"] = f("l2_att2").reshape(H, TOPO).T      # [15,4]
    # topo1 input to L2 = topo1_raw + l1_bias2; edge logit gets the const
    # twice (src+dst) -> fold 2*(b2@att2) into the dst-side rows only
    w["ta2c"] = (2.0 * (f("l1_bias2") @ w["att2T2"])).reshape(1, -1)   # [1,4]
    # heads (l2_bias folded into first-layer bias)
    b2f = f("l2_bias")
    for nm in ("v", "a"):
        w[f"{nm}w1"] = f(f"{nm}_w1")           # [512,128]
        w[f"{nm}b1"] = (f(f"{nm}_b1") + b2f @ f(f"{nm}_w1")).reshape(1, -1)
        w[f"{nm}w2"] = f(f"{nm}_w2")           # [128,1]
        w[f"{nm}b2"] = f(f"{nm}_b2").reshape(1, 1)
    # att2T2 flattened (h,j) row for the drain's ta2 reduce + const
    w["att2f"] = w["att2T2"].T.reshape(1, -1)  # [1,60] (h-major)
    return {k: v.astype(BF) for k, v in w.items()}


# --------------------------------------------------------------------------
# program builder
# --------------------------------------------------------------------------

class Prog:
    pass


def build_program(cfg, schedule, tn, debug=False):
    es = ExitStack()
    nc = bacc.Bacc("TRN2", target_bir_lowering=False, debug=False,
                   num_devices=cfg.CORES)
    pr = Prog()
    pr.nc = nc
    N, FIN, HID, TOPO, H, HC, ROW, NPC, NBLK = (
        cfg.N, cfg.FIN, cfg.HID, cfg.TOPO, cfg.H, cfg.HC, cfg.ROW, cfg.NPC,
        cfg.NBLK)
    TTOT = sum(schedule)
    W16 = TTOT * P // 16
    groups = [list(range(cfg.CORES))]
    blocks = ktiles(NPC)
    fkt = ktiles(FIN)
    ckt = ktiles(HC)
    offs = np.concatenate([[0], np.cumsum(schedule)]).astype(int)

    def din(name, shape, dtype=BF16):
        return nc.dram_tensor(name, list(shape), dtype, kind="ExternalInput")

    # ---- external inputs ----
    xT = din("xT_slice", (FIN, NPC))
    wnames = [("tw1", (FIN, HID)), ("tb1", (1, HID)), ("tw2", (HID, TOPO)),
              ("tb2", (1, TOPO)), ("wl1", (FIN, HC)), ("bl1", (1, HC)),
              ("A1", (FIN, 2 * H)), ("bA1", (1, 2 * H)), ("att2T1", (TOPO, H)),
              ("wl2", (HC, HC)), ("bl2", (1, HC)), ("att2T2", (TOPO, H)),
              ("ta2c", (1, H)), ("att2f", (1, H * TOPO)),
              ("vw1", (HC, HID)), ("vb1", (1, HID)), ("vw2", (HID, 1)),
              ("vb2", (1, 1)),
              ("aw1", (HC, HID)), ("ab1", (1, HID)), ("aw2", (HID, 1)),
              ("ab2", (1, 1))]
    W = {nm: din(nm, sh) for nm, sh in wnames}
    src_i = din("src_idx", (P, W16), I16)
    stt_i = din("stt_tab", (P, TTOT * 2 * P))

    # ---- outputs ----
    val_o = nc.dram_tensor("valence", [1, 1], F32, kind="ExternalOutput")
    aro_o = nc.dram_tensor("arousal", [1, 1], F32, kind="ExternalOutput")
    dbg = {}
    if debug:
        for nm, sh in [("dbg_h1", (P, HC)), ("dbg_tt", (P, TOPO + H)),
                       ("dbg_psm1", (P, 68)), ("dbg_h2", (P, HC)),
                       ("dbg_pool", (P, H)), ("dbg_aux", (P, 24)),
                       ("dbg_psm2", (P, H)), ("dbg_pd1", (P, 2 * H))]:
            dbg[nm] = nc.dram_tensor(nm, list(sh), F32, kind="ExternalOutput")

    # ---- internal DRAM ----
    ext_sl = [nc.dram_tensor(f"ext_slice{L}", [NPC, ROW], F8)
              for L in (1, 2)]
    ext_fl = [nc.dram_tensor(f"ext_full{L}", [N, ROW], F8,
                             addr_space="Shared") for L in (1, 2)]
    ext_pr = [nc.dram_tensor(f"ext_pair{L}", [N, ROW], F8,
                             addr_space="Shared") for L in (1, 2)]
    bar_io = [(nc.dram_tensor(f"bar_in{L}", [1, 1], F32),
               nc.dram_tensor(f"bar_out{L}", [2, 1], F32)) for L in (1, 2)]
    pgroups = [[2 * k, 2 * k + 1] for k in (0, 1, 2, 3)]
    pair_w = {1: [], 2: []}
    bar_inst = {}
    pool_in = nc.dram_tensor("pool_in", [1, HC], F32)
    pool_out = nc.dram_tensor("pool_out", [1, HC], F32, addr_space="Shared")

    with tile.TileContext(nc) as tc:
        # ================= static SBUF =================
        ident = nc.alloc_sbuf_tensor("ident", [P, P], F32).ap()
        make_identity(nc, ident)
        ones_row = nc.alloc_sbuf_tensor("ones_row", [1, NPC], BF16).ap()
        nc.gpsimd.memset(ones_row, 1.0)
        ones_col = nc.alloc_sbuf_tensor("ones_col", [P, 1], BF16).ap()
        nc.gpsimd.memset(ones_col, 1.0)
        eps_col = nc.alloc_sbuf_tensor("eps_col", [P, 1], F32).ap()
        nc.gpsimd.memset(eps_col, 1e-30)

        src_sb = nc.alloc_sbuf_tensor("src_sb", [P, W16], I16).ap()
        nc.sync.dma_start(src_sb, src_i[:, :])

        # resident activations / weights
        xT_sb = [nc.alloc_sbuf_tensor(f"xT{i}", [P, NPC], BF16).ap()
                 for i in range(len(fkt))]
        for i, (fo, fk) in enumerate(fkt):
            nc.sync.dma_start(xT_sb[i][:fk, :], xT[fo:fo + fk, :])
        hfmT = [nc.alloc_sbuf_tensor(f"hfmT{i}", [P, NPC], BF16).ap()
                for i in range(len(ckt))]
        topoT0 = nc.alloc_sbuf_tensor("topoT0", [TOPO, NPC], BF16).ap()
        datt1 = nc.alloc_sbuf_tensor("datt1", [P, NBLK * 2 * H], BF16).ap()
        datt2 = nc.alloc_sbuf_tensor("datt2", [P, NBLK * H], BF16).ap()
        nc.vector.memset(datt1, 0.0)   # rows past a partial block stay 0
        nc.vector.memset(datt2, 0.0)
        # near-pass partial aggregates (bf16 so the merge can be a matmul)
        partF = nc.alloc_sbuf_tensor("partF", [P, NBLK * HC], BF16).ap()
        partS = nc.alloc_sbuf_tensor("partS", [P, NBLK * (16 * H + H)],
                                     BF16).ap()
        pid = nc.partition_id()

        wsb = {}
        for nm, sh in wnames:
            if sh[0] <= P:
                wsb[nm] = nc.alloc_sbuf_tensor(f"w_{nm}", list(sh), BF16).ap()
                nc.sync.dma_start(wsb[nm], W[nm][:, :])
            else:  # k-tiled along the first (contraction) dim
                tiles = []
                for i, (fo, fk) in enumerate(ktiles(sh[0])):
                    t = nc.alloc_sbuf_tensor(f"w_{nm}{i}", [fk, sh[1]],
                                             BF16).ap()
                    nc.sync.dma_start(t, W[nm][fo:fo + fk, :])
                    tiles.append(t)
                wsb[nm] = tiles
        # att2f / ta2c broadcast to all partitions
        att2bc = nc.alloc_sbuf_tensor("att2bc", [P, H * TOPO], BF16).ap()
        nc.gpsimd.partition_broadcast(att2bc, wsb["att2f"][0:1, :])
        ta2cbc = nc.alloc_sbuf_tensor("ta2cbc", [P, H], BF16).ap()
        nc.gpsimd.partition_broadcast(ta2cbc, wsb["ta2c"][0:1, :])
        ident_bf = nc.alloc_sbuf_tensor("ident_bf", [P, P], BF16).ap()
        nc.vector.tensor_copy(ident_bf, ident)

        # ================= phase A: topo MLP + L1 prep =================
        with tc.tile_pool(name="ppA", bufs=1, space="PSUM") as ppA, \
             tc.tile_pool(name="ppA2", bufs=2, space="PSUM") as ppA2, \
             tc.tile_pool(name="cpA", bufs=3) as cpA, \
             tc.tile_pool(name="spA", bufs=2) as spA:
            # --- topo extractor MLP (feat-major: out rows = hid/topo) ---
            NG = 512
            for go in range(0, NPC, NG):
                gs = min(NG, NPC - go)
                ph = ppA.tile([P, NG], F32, tag="ph", name="ph", space="PSUM")
                for i, (fo, fk) in enumerate(fkt):
                    nc.tensor.matmul(ph[:, :gs], lhsT=wsb["tw1"][i][:fk, :],
                                     rhs=xT_sb[i][:fk, go:go + gs],
                                     start=i == 0, stop=False,
                                     skip_group_check=True)
                nc.tensor.matmul(ph[:, :gs], lhsT=wsb["tb1"][:, :],
                                 rhs=ones_row[:, go:go + gs], start=False,
                                 stop=True, skip_group_check=True)
                t_hid = spA.tile([P, NG], BF16, tag="t_hid", name="t_hid")
                nc.scalar.activation(t_hid[:, :gs], ph[:, :gs], AF.Relu)
                pt = ppA.tile([TOPO, NG], F32, tag="pt", name="pt", space="PSUM")
                nc.tensor.matmul(pt[:, :gs], lhsT=wsb["tw2"][:, :],
                                 rhs=t_hid[:, :gs], start=True, stop=False,
                                 skip_group_check=True)
                nc.tensor.matmul(pt[:, :gs], lhsT=wsb["tb2"][:, :],
                                 rhs=ones_row[:, go:go + gs], start=False,
                                 stop=True, skip_group_check=True)
                nc.vector.tensor_copy(topoT0[:, go:go + gs], pt[:, :gs])

            # --- L1 prep per block ---
            for bi, (bo, bs) in enumerate(blocks):
                pm = ppA2.tile([P, HC], F32, tag="pm", name="pm", space="PSUM")
                pa = ppA.tile([P, 2 * H], F32, tag="pa", name="pa", space="PSUM")
                for i, (fo, fk) in enumerate(fkt):
                    nc.tensor.matmul(pm[:bs, :], lhsT=xT_sb[i][:fk, bo:bo + bs],
                                     rhs=wsb["wl1"][i][:fk, :],
                                     start=i == 0, stop=False,
                                     skip_group_check=True)
                    nc.tensor.matmul(pa[:bs, :], lhsT=xT_sb[i][:fk, bo:bo + bs],
                                     rhs=wsb["A1"][i][:fk, :],
                                     start=i == 0, stop=False,
                                     skip_group_check=True)
                nc.tensor.matmul(pm[:bs, :], lhsT=ones_row[:, bo:bo + bs],
                                 rhs=wsb["bl1"][:, :], start=False, stop=True,
                                 skip_group_check=True)
                nc.tensor.matmul(pa[:bs, :], lhsT=ones_row[:, bo:bo + bs],
                                 rhs=wsb["bA1"][:, :], start=False, stop=True,
                                 skip_group_check=True)
                pta = ppA.tile([P, H], F32, tag="pta", name="pta", space="PSUM")
                nc.tensor.matmul(pta[:bs, :], lhsT=topoT0[:, bo:bo + bs],
                                 rhs=wsb["att2T1"][:, :], start=True,
                                 stop=True, skip_group_check=True)
                ptt = ppA.tile([P, TOPO], BF16, tag="ptt", name="ptt",
                               space="PSUM")
                nc.tensor.transpose(ptt[:bs, :TOPO],
                                    topoT0[:, bo:bo + bs],
                                    ident_bf[:TOPO, :TOPO])
                ext = cpA.tile([P, ROW], F8, tag="ext", name="ext")
                extb = ext[:, :].bitcast(BF16)
                nc.scalar.copy(ext[:bs, 0:HC], pm[:bs, :])
                nc.scalar.copy(extb[:bs, cfg.C_TOPO:cfg.C_TOPO + TOPO],
                               ptt[:bs, :TOPO])
                nc.vector.memset(extb[:bs, cfg.C_ONE:cfg.C_ONE + 1], 1.0)
                nc.scalar.copy(extb[:bs, cfg.C_AL:cfg.C_AL + H], pa[:bs, 0:H])
                nc.scalar.copy(extb[:bs, cfg.C_TA:cfg.C_TA + H], pta[:bs, :])
                nc.sync.dma_start(ext_sl[0][bo:bo + bs, :], ext[:bs, :])
                wpr = nc.sync.dma_start(
                    ext_pr[0][:, :].rearrange("(c n) r -> c n r",
                                              c=cfg.CORES)[pid][bo:bo + bs, :],
                    ext[:bs, :])
                pair_w[1].append(wpr.ins)
                if debug and bi == 0:
                    da = cpA.tile([P, 24], F32, tag="dbga", name="dbga")
                    nc.vector.tensor_copy(da[:, :],
                                          extb[:, cfg.C_TOPO:cfg.C_TOPO + 24])
                    nc.sync.dma_start(dbg["dbg_aux"][:, :], da[:, :])
                # dst-side rows: [ar | ta]
                nc.vector.tensor_copy(datt1[:bs, bi * 2 * H:bi * 2 * H + H],
                                      pa[:bs, H:2 * H])
                nc.vector.tensor_copy(
                    datt1[:bs, bi * 2 * H + H:(bi + 1) * 2 * H], pta[:bs, :])
            pass  # collectives for L1 are emitted inside the phase-B scope

        # ================= edge phase (shared emitter) =================
        TMAX = max(schedule)

        def emit_collectives(L):
            # cheap pair barrier, PINNED before the AllGather so the
            # near pass (which waits only on the barrier) overlaps the AG
            bar = nc.gpsimd.collective_compute(
                "AllGather", OP.bypass, replica_groups=pgroups,
                ins=[bar_io[L - 1][0][:, :]], outs=[bar_io[L - 1][1][:, :]])
            for w in pair_w[L]:
                add_dep_helper(bar.ins, w, reason="pair barrier")
            bar_inst[L] = bar
            ag = nc.gpsimd.collective_compute(
                "AllGather", OP.bypass, replica_groups=groups,
                ins=[ext_sl[L - 1][:, :]], outs=[ext_fl[L - 1][:, :]])
            add_dep_helper(ag.ins, bar.ins, reason="pin barrier before AG")

        SMW1 = 16 * H + H

        def emit_tiles(L, gp, sp, pp, pp2, bi, ta, te_, src_t, dep_i, mrg):
            """Tiles [ta, te_) of block bi -> (pf, psm) psums."""
            AUXW = 2 * H if L == 1 else H      # lg width per tile
            AUXO = cfg.C_AL if L == 1 else cfg.C_TA2
            base = int(offs[bi]) + ta
            nt = te_ - ta
            SMW = SMW1 if L == 1 else H
            # ---- gathers ----
            G = gp.tile([P, TMAX * ROW], F8, tag="G", name="G")
            for go in range(0, nt, GT_MAX):
                gn = min(GT_MAX, nt - go)
                c0 = (base + go) * 8
                gi = nc.gpsimd.dma_gather(
                    G[:, go * ROW:(go + gn) * ROW].rearrange(
                        "p (t e) -> p t e", e=ROW),
                    src_t[:, :], src_sb[:, c0:c0 + 8 * gn],
                    num_idxs=P * gn, num_idxs_reg=P * gn, elem_size=ROW,
                    queue_num=0)
                if dep_i is not None:
                    add_dep_helper(gi.ins, dep_i.ins,
                                   reason="near gather after pair barrier")
            # ---- St / StT (host-built one-hot tables) ----
            stt = sp.tile([P, TMAX * 2 * P], BF16, tag="stt", name="stt")
            nc.sync.dma_start(stt[:, 0:nt * 2 * P],
                              stt_i[:, base * 2 * P:(base + nt) * 2 * P])

            def St(t):
                return stt[:, t * 2 * P:t * 2 * P + P]

            def StT(t):
                return stt[:, t * 2 * P + P:(t + 1) * 2 * P]
            # ---- dst-logit lookup ----
            pD = pp.tile([P, TMAX * AUXW], F32, tag="pD", name="pD",
                         space="PSUM")
            dsl = (datt1[:, bi * 2 * H:(bi + 1) * 2 * H] if L == 1
                   else datt2[:, bi * H:(bi + 1) * H])
            for t in range(nt):
                nc.tensor.matmul(pD[:, t * AUXW:(t + 1) * AUXW],
                                 lhsT=StT(t), rhs=dsl, start=True,
                                 stop=True, skip_group_check=True)
            # ---- batched logits ----
            Gb = G[:, 0:nt * ROW].bitcast(BF16).rearrange(
                "p (t e) -> p t e", e=cfg.RB)
            lg = sp.tile([P, TMAX * AUXW], F32, tag="lg", name="lg")
            nc.vector.tensor_tensor(
                lg[:, 0:nt * AUXW].rearrange("p (t c) -> p t c", c=AUXW),
                Gb[:, :, AUXO:AUXO + AUXW],
                pD[:, 0:nt * AUXW].rearrange("p (t c) -> p t c", c=AUXW),
                OP.add)
            lr = sp.tile([P, TMAX * AUXW], F32, tag="lr", name="lr")
            nc.vector.scalar_tensor_tensor(
                lr[:, 0:nt * AUXW], lg[:, 0:nt * AUXW], cfg.NEG,
                lg[:, 0:nt * AUXW], OP.mult, OP.max)
            et = sp.tile([P, TMAX * AUXW], BF16, tag="et", name="et")
            nc.scalar.activation(et[:, 0:nt * AUXW], lr[:, 0:nt * AUXW],
                                 AF.Exp)
            etv = et[:, 0:nt * AUXW].rearrange("p (t c) -> p t c", c=AUXW)
            # ---- weighted messages (split DVE / gpsimd) ----
            Gp = gp.tile([P, TMAX * HC], BF16, tag="Gp", name="Gp")
            e2off = H if L == 1 else 0
            Gf = G[:, 0:nt * ROW].rearrange("p (t e) -> p t e", e=ROW)
            pct = GP_POOL_PCT if L == 1 else GP_POOL_PCT2
            ks = (nt * pct + 99) // 100      # first ks tiles on Pool

            def gp_op(eng, t0, t1):
                if t1 <= t0:
                    return
                eng.tensor_tensor(
                    Gp[:, t0 * HC:t1 * HC].rearrange(
                        "p (t h c) -> p t h c", h=H, c=HID),
                    Gf[:, t0:t1, 0:HC].rearrange(
                        "p t (h c) -> p t h c", c=HID),
                    etv[:, t0:t1, e2off:e2off + H].unsqueeze(
                        3).to_broadcast((P, t1 - t0, H, HID)),
                    OP.mult)
            for t0 in range(0, ks, 3):        # gpsimd in 3-tile chunks
                gp_op(nc.gpsimd, t0, min(t0 + 3, ks))
            for t0 in range(ks, nt, 4):       # DVE in 4-tile chunks
                gp_op(nc.vector, t0, min(t0 + 4, nt))
            if L == 1:
                SMe = sp.tile([P, TMAX * SMW], BF16, tag="SMe", name="SMe")
                SMv = SMe[:, 0:nt * SMW].rearrange("p (t c) -> p t c", c=SMW)
                nc.vector.tensor_tensor(
                    SMv[:, :, 0:16 * H].rearrange(
                        "p t (h j) -> p t h j", j=16),
                    Gb[:, :, cfg.C_TOPO:cfg.C_TOPO + 16].unsqueeze(
                        2).to_broadcast((P, nt, H, 16)),
                    etv[:, :, 0:H].unsqueeze(3).to_broadcast(
                        (P, nt, H, 16)),
                    OP.mult)
                nc.scalar.copy(SMv[:, :, 16 * H:SMW], etv[:, :, H:2 * H])
            else:
                SMe = et
            # ---- aggregation matmuls (mrg: preload stashed partials) ----
            pf = pp2.tile([P, HC], F32, tag="pf", name="pf", space="PSUM")
            psm = pp2.tile([P, SMW], F32, tag="psm", name="psm",
                           space="PSUM")
            if mrg:
                nc.tensor.matmul(pf[:, :], lhsT=ident_bf,
                                 rhs=partF[:, bi * HC:(bi + 1) * HC],
                                 start=True, stop=False,
                                 skip_group_check=True)
                nc.tensor.matmul(psm[:, :], lhsT=ident_bf,
                                 rhs=partS[:, bi * SMW1:bi * SMW1 + SMW],
                                 start=True, stop=False,
                                 skip_group_check=True)
            for t in range(nt):
                st0 = (t == 0) and not mrg
                sp1 = t == nt - 1
                nc.tensor.matmul(pf[:, :], lhsT=St(t),
                                 rhs=Gp[:, t * HC:(t + 1) * HC],
                                 start=st0, stop=sp1, skip_group_check=True)
                nc.tensor.matmul(psm[:, :], lhsT=St(t),
                                 rhs=SMe[:, t * SMW:(t + 1) * SMW],
                                 start=st0, stop=sp1, skip_group_check=True)
            return pf, psm

        def emit_edge(L, gp, sp, pp, pp2):
            SMW = SMW1 if L == 1 else H
            # near pass: pair-local tiles, overlapping the AllGather
            for bi in range(len(blocks)):
                if tn[bi] <= 0:
                    continue
                pf, psm = emit_tiles(L, gp, sp, pp, pp2, bi, 0, tn[bi],
                                     ext_pr[L - 1], bar_inst[L], False)
                nc.scalar.copy(partF[:, bi * HC:(bi + 1) * HC], pf[:, :])
                nc.scalar.copy(partS[:, bi * SMW1:bi * SMW1 + SMW],
                               psm[:, :])
            # far pass after the AllGather, merging the stash
            for bi, (bo, bs) in enumerate(blocks):
                pf, psm = emit_tiles(L, gp, sp, pp, pp2, bi, tn[bi],
                                     schedule[bi], ext_fl[L - 1], None,
                                     tn[bi] > 0)
                if L == 1:
                    drain1(bi, bo, bs, pf, psm, sp, pp, pp2)
                else:
                    drain2(bi, bo, bs, pf, psm, sp, pp)

        # ---- L1 drain + fused L2 prep ----
        def drain1(bi, bo, bs, pf, psm, sp, pp, pp2):
            # rec2 = 1/sum(e2), rec1' = 1/(H*sum(e1))
            den = sp.tile([P, 2 * H], F32, tag="den", name="den")
            nc.vector.tensor_scalar(
                den[:, 0:H].unsqueeze(2),
                psm[:, 0:16 * H].rearrange("p (h j) -> p h j", j=16)[
                    :, :, 15:16],
                float(H), eps_col[:, 0:1], OP.mult, OP.max)
            nc.vector.tensor_tensor(den[:, H:2 * H], psm[:, 16 * H:16 * H + H],
                                    eps_col[:, 0:1].to_broadcast((P, H)),
                                    OP.max)
            rec = sp.tile([P, 2 * H], F32, tag="rec", name="rec")
            nc.vector.reciprocal(rec[:, :], den[:, :])
            # h1 = agg_feat * rec2 (node-major, bf16; per-head scale on Act)
            h1 = sp.tile([P, HC], BF16, tag="h1", name="h1")
            for h in range(H):
                nc.scalar.activation(h1[:, h * HID:(h + 1) * HID],
                                     pf[:, h * HID:(h + 1) * HID], AF.Copy,
                                     scale=rec[:, H + h:H + h + 1])
            # topo1_raw = sum_h agg_topo_h * rec1'   [d, 15]
            tp = sp.tile([P, TOPO * H], F32, tag="tp", name="tp")
            nc.vector.tensor_tensor(
                tp[:, :].rearrange("p (j h) -> p j h", h=H),
                psm[:, 0:16 * H].rearrange("p (h j) -> p h j", j=16)[
                    :, :, 0:TOPO].transpose([0, 2, 1]),
                rec[:, 0:H].unsqueeze(1).to_broadcast((P, TOPO, H)),
                OP.mult)
            t1 = sp.tile([P, TOPO], F32, tag="t1", name="t1")
            nc.vector.tensor_reduce(
                t1[:, :], tp[:, :].rearrange("p (j h) -> p j h", h=H),
                mybir.AxisListType.X, OP.add)
            # ta2 = topo1_raw @ att2T2 (per-node, via DVE reduce)
            tq = sp.tile([P, H * TOPO], F32, tag="tq", name="tq")
            nc.vector.tensor_tensor(
                tq[:, :].rearrange("p (h j) -> p h j", j=TOPO),
                t1[:, :].unsqueeze(1).to_broadcast((P, H, TOPO)),
                att2bc[:, :].rearrange("p (h j) -> p h j", j=TOPO),
                OP.mult)
            ta2 = sp.tile([P, H], F32, tag="ta2", name="ta2")
            nc.vector.tensor_reduce(
                ta2[:, :], tq[:, :].rearrange("p (h j) -> p h j", j=TOPO),
                mybir.AxisListType.X, OP.add)
            # dst rows for L2: ta2 + 2*(b2@att2)
            nc.vector.tensor_tensor(datt2[:bs, bi * H:(bi + 1) * H],
                                    ta2[:bs, :],
                                    ta2cbc[:bs, :], OP.add)
            if debug and bi == 0:
                dh = sp.tile([P, HC], F32, tag="dbgh", name="dbgh")
                nc.vector.tensor_copy(dh[:, :], h1[:, :])
                nc.sync.dma_start(dbg["dbg_h1"][:, :], dh[:, :])
                dtt = sp.tile([P, TOPO + H], F32, tag="dbgt", name="dbgt")
                nc.vector.tensor_copy(dtt[:, 0:TOPO], t1[:, :])
                nc.vector.tensor_copy(dtt[:, TOPO:TOPO + H], ta2[:, :])
                nc.sync.dma_start(dbg["dbg_tt"][:, :], dtt[:, :])
            # transpose h1 -> hfmT tiles
            for ci, (co, ck) in enumerate(ckt):
                ptr = pp.tile([P, P], BF16, tag="ptr", name="ptr",
                              space="PSUM")
                nc.tensor.transpose(ptr[:ck, :bs], h1[:bs, co:co + ck],
                                    ident_bf[:bs, :bs])
                nc.scalar.copy(hfmT[ci][:ck, bo:bo + bs], ptr[:ck, :bs])
            # ---- fused L2 prep for this block ----
            pm2 = pp2.tile([P, HC], F32, tag="pm2", name="pm2", space="PSUM")
            for ci, (co, ck) in enumerate(ckt):
                nc.tensor.matmul(pm2[:bs, :], lhsT=hfmT[ci][:ck, bo:bo + bs],
                                 rhs=wsb["wl2"][ci][:ck, :],
                                 start=ci == 0, stop=False,
                                 skip_group_check=True)
            nc.tensor.matmul(pm2[:bs, :], lhsT=ones_row[:, bo:bo + bs],
                             rhs=wsb["bl2"][:, :], start=False, stop=True,
                             skip_group_check=True)
            ext = sp.tile([P, ROW], F8, tag="ext2", name="ext2")
            nc.scalar.copy(ext[:bs, 0:HC], pm2[:bs, :])
            nc.scalar.copy(ext[:, :].bitcast(BF16)[
                :bs, cfg.C_TA2:cfg.C_TA2 + H], ta2[:bs, :])
            nc.sync.dma_start(ext_sl[1][bo:bo + bs, :], ext[:bs, :])
            wpr = nc.sync.dma_start(
                ext_pr[1][:, :].rearrange("(c n) r -> c n r",
                                          c=cfg.CORES)[pid][bo:bo + bs, :],
                ext[:bs, :])
            pair_w[2].append(wpr.ins)

        # ---- L2 drain: normalize + pooled partial ----
        def drain2(bi, bo, bs, pf, psm, sp, pp):
            den = sp.tile([P, H], F32, tag="den2", name="den2")
            nc.vector.tensor_tensor(den[:, :], psm[:, 0:H],
                                    eps_col[:, 0:1].to_broadcast((P, H)),
                                    OP.max)
            rec = sp.tile([P, H], F32, tag="rec2", name="rec2")
            nc.vector.reciprocal(rec[:, :], den[:, :])
            h2 = sp.tile([P, HC], BF16, tag="h2", name="h2")
            for h in range(H):
                nc.scalar.activation(h2[:, h * HID:(h + 1) * HID],
                                     pf[:, h * HID:(h + 1) * HID], AF.Copy,
                                     scale=rec[:, h:h + 1])
            if debug and bi == 0:
                dh = sp.tile([P, HC], F32, tag="dbgh2", name="dbgh2")
                nc.vector.tensor_copy(dh[:, :], h2[:, :])
                nc.sync.dma_start(dbg["dbg_h2"][:, :], dh[:, :])
            nc.tensor.matmul(pr.pool_ps[:, :], lhsT=ones_col[:bs, 0:1],
                             rhs=h2[:bs, :], start=bi == 0,
                             stop=bi == len(blocks) - 1,
                             skip_group_check=True)

        # ================= phase B/C: L1 edges (+L2 prep) =================
        # PSUM banks: (pf+psm) 2x2 + pm2 2 + pD/ptr 1 each = 8 of 8
        with tc.tile_pool(name="gpB", bufs=2) as gpB, \
             tc.tile_pool(name="spB", bufs=3) as spB, \
             tc.tile_pool(name="ppB", bufs=1, space="PSUM") as ppB, \
             tc.tile_pool(name="ppB2", bufs=2, space="PSUM") as ppB2:
            emit_collectives(1)
            emit_edge(1, gpB, spB, ppB, ppB2)

        # ================= phase D: L2 edges =================
        with tc.tile_pool(name="gpD", bufs=2) as gpD, \
             tc.tile_pool(name="spD", bufs=3) as spD, \
             tc.tile_pool(name="ppD", bufs=1, space="PSUM") as ppD, \
             tc.tile_pool(name="ppD2", bufs=2, space="PSUM") as ppD2, \
             tc.tile_pool(name="plD", bufs=1, space="PSUM") as plD:
            pr.pool_ps = plD.tile([1, HC], F32, tag="pool", name="pool",
                                  space="PSUM", bufs=1)
            emit_collectives(2)
            emit_edge(2, gpD, spD, ppD, ppD2)

            # ---- pool + heads ----
            pooled = spD.tile([1, HC], F32, tag="pooled", name="pooled")
            nc.vector.tensor_copy(pooled[:, :], pr.pool_ps[:, :])
            nc.sync.dma_start(pool_in[:, :], pooled[:, :])
            nc.gpsimd.collective_compute(
                "AllReduce", OP.add, replica_groups=groups,
                ins=[pool_in[:, :]], outs=[pool_out[:, :]])
            # load back column-major: pmean_cols[c, h] = pool_out[h*HID+c]
            pooled2 = spD.tile([P, H], F32, tag="pooled2", name="pooled2")
            with nc.allow_non_contiguous_dma("pool row -> col-major reload"):
                nc.sync.dma_start(
                    pooled2[:, :],
                    pool_out[:, :].rearrange("o (h c) -> (o c) h", c=HID))
            if debug:
                nc.sync.dma_start(dbg["dbg_pool"][:, :], pooled2[:, :])
            pmean = spD.tile([P, H], BF16, tag="pmean", name="pmean")
            nc.vector.tensor_scalar(pmean[:, :], pooled2[:, :], 1.0 / N,
                                    None, OP.mult)
            for nm, out_t in (("v", val_o), ("a", aro_o)):
                pm = ppD.tile([P, 1], F32, tag="mlp", name="mlp", space="PSUM")
                for ki in range(H):
                    nc.tensor.matmul(pm[:, :], lhsT=wsb[f"{nm}w1"][ki][:, :],
                                     rhs=pmean[:, ki:ki + 1], start=ki == 0,
                                     stop=False, skip_group_check=True)
                nc.tensor.matmul(pm[:, :], lhsT=wsb[f"{nm}b1"][:, :],
                                 rhs=ones_col[0:1, :], start=False, stop=True,
                                 skip_group_check=True)
                hv = spD.tile([P, 1], BF16, tag=f"{nm}hv", name=f"{nm}hv")
                nc.scalar.activation(hv[:, :], pm[:, :], AF.Relu)
                po = ppD.tile([1, 1], F32, tag="mlpo", name="mlpo",
                              space="PSUM")
                nc.tensor.matmul(po[:, :], lhsT=hv[:, :],
                                 rhs=wsb[f"{nm}w2"][:, :], start=True,
                                 stop=False, skip_group_check=True)
                nc.tensor.matmul(po[:, :], lhsT=wsb[f"{nm}b2"][:, :],
                                 rhs=ones_col[0:1, :], start=False, stop=True,
                                 skip_group_check=True)
                ov = spD.tile([1, 1], F32, tag=f"{nm}ov", name=f"{nm}ov")
                nc.vector.tensor_copy(ov[:, :], po[:, :])
                nc.sync.dma_start(out_t[:, :], ov[:, :])

    nc.compile()
    es.close()
    return pr


# --------------------------------------------------------------------------
# entry point
# --------------------------------------------------------------------------

_CACHE = {}


def make_in_maps(inputs, cfg, src_w, stt):
    x = np.asarray(inputs["x"], dtype=np.float32)
    shared = host_weights(inputs, cfg)
    in_maps = []
    for c in range(cfg.CORES):
        m = dict(shared)
        m["xT_slice"] = np.ascontiguousarray(
            x[c * cfg.NPC:(c + 1) * cfg.NPC].T.astype(BF))
        m["src_idx"] = np.ascontiguousarray(src_w[c])
        m["stt_tab"] = stt[c]
        in_maps.append(m)
    return in_maps


def run(inputs, cfg=CFG, trace=False):
    schedule, src_w, stt, tn = host_prep(inputs["edge_index"], cfg)
    key = (cfg.N, cfg.E, tuple(schedule), tuple(tn))
    if key not in _CACHE:
        _CACHE[key] = build_program(cfg, schedule, tn)
    pr = _CACHE[key]
    in_maps = make_in_maps(inputs, cfg, src_w, stt)
    res = run_bass_kernel_spmd(pr.nc, in_maps, list(range(cfg.CORES)),
                               trace=trace)
    out = res.results[0]
    return (np.asarray(out["valence"], np.float32),
            np.asarray(out["arousal"], np.float32)), res


def kernel(**inputs):
    (val, aro), _ = run(inputs)
    return (val, aro)
